# revision 1
# baseline (speedup 1.0000x reference)
"""GCN block (2-layer GCNConv + ReLU) on 8 Trainium2 NeuronCores.

Strategy (1D node partitioning per the sharding hint):
  - Core c owns target nodes [c*N/8, (c+1)*N/8) and every edge whose target
    (col) lands there.
  - Aggregation is reordered before the weight matmul: A_norm @ (x W) ==
    (A_norm @ x) W, so each layer gathers raw table rows, segment-sums them
    into 128-target-node blocks, then applies the dense 96x96 weights.
  - Segment-sum runs on the tensor engine: for each 128-edge chunk of the
    target-sorted edge stream, a selection matrix S[e, m] = norm[e] *
    (localcol[e] == m) is built on the vector engine with one dual-op
    tensor_scalar (is_equal then mult against an iota tile), and
    psum[128 targets, 96] += S.T @ M accumulates over the block's chunks.
    Chunks are packed densely (they may straddle block boundaries; each
    (chunk, block) segment gets its own S/matmul with zeros off-segment).
  - Self-loops of full blocks skip the gather: their table rows are a
    contiguous load and a diagonal S carries dinv^2.
  - Messages M are gathered from a fp16 [N, 96] table in DRAM with
    per-chunk indirect DMAs ([128,1] per-partition offsets - the only
    indirect form this runtime supports).
  - Layer 1 computes T2 = relu((A@x)W1 + b1) @ W2 for owned nodes
    (W2 folded in while the data is feature-major), then an 8-core
    AllGather rebuilds the full table for layer 2's gathers.
  - Layer 2 is aggregation + bias + relu only, written node-major.
"""

import os
import sys

for _p in ("/opt/trn_rl_repo", "/root/.axon_site/_ro/trn_rl_repo"):
    if os.path.isdir(_p) and _p not in sys.path:
        sys.path.insert(0, _p)

import numpy as np

import concourse.bass as bass
import concourse.bacc as bacc
import concourse.mybir as mybir
import concourse.tile as tile
from concourse import bass_utils

F16 = mybir.dt.float16
F32 = mybir.dt.float32
I32 = mybir.dt.int32

P = 128          # partitions / edges per chunk / nodes per target block
D = 96           # feature dim
NCORES = 8


def _preprocess(row, col, ew, N):
    """Bucket edges by owning core, sort by target, pack densely into
    128-edge chunks shared across cores (per-block counts padded to the
    max over cores so one SPMD program fits all eight).

    Returns per-core gather/selection metadata plus the segment schedule
    (chunk, block, first, last) that drives program generation.
    """
    npc = N // NCORES
    nblk = (npc + P - 1) // P
    nfull = npc // P          # blocks whose self-loops use the direct path

    deg = np.bincount(col, weights=ew, minlength=N) + 1.0
    dinv = (1.0 / np.sqrt(deg)).astype(np.float32)
    norm = (dinv[row] * ew * dinv[col]).astype(np.float32)
    selfn = (dinv * dinv).astype(np.float32)

    # per-core edge lists sorted by local target; self-loops only for the
    # partial tail block (full blocks handle them without a gather)
    cores = []
    counts_all = []
    nb = np.zeros(nblk, dtype=np.int64)
    tail = npc - nfull * P
    for c in range(NCORES):
        lo, hi = c * npc, (c + 1) * npc
        m = (col >= lo) & (col < hi)
        r = np.asarray(row[m], dtype=np.int64)
        cl = np.asarray(col[m] - lo, dtype=np.int64)
        w = norm[m]
        if tail:
            tn = np.arange(nfull * P, npc, dtype=np.int64)
            r = np.concatenate([r, tn + lo])
            cl = np.concatenate([cl, tn])
            w = np.concatenate([w, selfn[lo + tn]])
        order = np.argsort(cl, kind="stable")
        r, cl, w = r[order], cl[order], w[order]
        counts = np.bincount(cl // P, minlength=nblk)
        cores.append((r, cl, w))
        counts_all.append(counts)
        nb = np.maximum(nb, counts)

    L = int(nb.sum())
    nchunks = (L + P - 1) // P
    Lpad = nchunks * P
    nb_pad = nb.copy()
    nb_pad[-1] += Lpad - L      # stream tail padding charged to last block

    # block start positions in the padded stream, and the segment schedule
    starts = np.zeros(nblk + 1, dtype=np.int64)
    starts[1:] = np.cumsum(nb_pad)
    segs = []  # (chunk, block, first, last, lane_lo, lane_hi)
    for b in range(nblk):
        s, e = int(starts[b]), int(starts[b + 1])
        c0, c1 = s // P, (e - 1) // P
        for cch in range(c0, c1 + 1):
            lo_ = max(s, cch * P) - cch * P
            hi_ = min(e, (cch + 1) * P) - cch * P
            segs.append((cch, b, cch == c0, cch == c1, lo_, hi_))
    nseg = len(segs)

    rowidx = np.zeros((NCORES, P, nchunks), np.int32)
    colseg = np.zeros((NCORES, P, nseg), np.float32)
    wseg = np.zeros((NCORES, P, nseg), np.float32)
    selfw = np.zeros((NCORES, P, max(nfull, 1)), np.float32)
    for c in range(NCORES):
        r, cl, w = cores[c]
        counts = counts_all[c]
        # build the padded stream for this core
        sr = np.zeros(Lpad, np.int64)
        scl = np.zeros(Lpad, np.int64)
        sw = np.zeros(Lpad, np.float32)
        e0 = 0
        for b in range(nblk):
            n = int(counts[b])
            s = int(starts[b])
            sr[s:s + n] = r[e0:e0 + n]
            scl[s:s + n] = cl[e0:e0 + n] - b * P
            scl[s + n:int(starts[b + 1])] = 0
            sw[s:s + n] = w[e0:e0 + n]
            e0 += n
        rowidx[c] = sr.reshape(nchunks, P).T
        scl2 = scl.reshape(nchunks, P).T
        sw2 = sw.reshape(nchunks, P).T
        for si, (cch, b, _f, _l, lo_, hi_) in enumerate(segs):
            colseg[c, lo_:hi_, si] = scl2[lo_:hi_, cch]
            wseg[c, lo_:hi_, si] = sw2[lo_:hi_, cch]
        lo = c * npc
        for b in range(nfull):
            selfw[c, :, b] = selfn[lo + b * P: lo + (b + 1) * P]

    return (rowidx, colseg, wseg, selfw, segs, nchunks, npc, nblk, nfull)


def _build_program(N, npc, nblk, nfull, nchunks, segs, repeat=1,
                   no_coll=False, mode='full'):
    nseg = len(segs)
    nc = bacc.Bacc("TRN2", target_bir_lowering=False, debug=False,
                   enable_asserts=False, num_devices=NCORES)

    t1 = nc.dram_tensor("t1", [N, D], F16, kind="ExternalInput").ap()
    xo_d = nc.dram_tensor("x_own", [nblk * P, D], F16, kind="ExternalInput").ap()
    rowidx_d = nc.dram_tensor("rowidx", [P, nchunks], I32, kind="ExternalInput").ap()
    colseg_d = nc.dram_tensor("colseg", [P, nseg], F32, kind="ExternalInput").ap()
    wseg_d = nc.dram_tensor("wseg", [P, nseg], F32, kind="ExternalInput").ap()
    selfw_d = nc.dram_tensor("selfw", [P, max(nfull, 1)], F32,
                             kind="ExternalInput").ap()
    iota_d = nc.dram_tensor("iota", [P, P], F16, kind="ExternalInput").ap()
    iotac_d = nc.dram_tensor("iotac", [P, 1], F32, kind="ExternalInput").ap()
    iotaf_d = nc.dram_tensor("iotaf", [P, P], F32, kind="ExternalInput").ap()
    ident_d = nc.dram_tensor("ident", [P, P], F16, kind="ExternalInput").ap()
    w1_d = nc.dram_tensor("w1", [D, D], F16, kind="ExternalInput").ap()
    w2_d = nc.dram_tensor("w2", [D, D], F16, kind="ExternalInput").ap()
    b1_d = nc.dram_tensor("b1", [D, 1], F32, kind="ExternalInput").ap()
    b2rep_d = nc.dram_tensor("b2rep", [P, D], F32, kind="ExternalInput").ap()
    out_d = nc.dram_tensor("out", [nblk * P, D], F32, kind="ExternalOutput").ap()

    with tile.TileContext(nc) as tc:
        with (
            tc.tile_pool(name="const", bufs=1) as const_pool,
            tc.tile_pool(name="meta", bufs=1) as meta_pool,
            tc.tile_pool(name="gath", bufs=6) as g_pool,
            tc.tile_pool(name="smat", bufs=16) as s_pool,
            tc.tile_pool(name="sbig", bufs=4) as sbig_pool,
            tc.tile_pool(name="work", bufs=2) as w_pool,
            tc.tile_pool(name="own", bufs=2) as own_pool,
            tc.tile_pool(name="pagg", bufs=3, space="PSUM") as pagg_pool,
            tc.tile_pool(name="pmisc", bufs=1, space="PSUM") as pmisc_pool,
            tc.tile_pool(name="dram", bufs=1, space="DRAM") as dram_pool,
        ):
            iota_sb = const_pool.tile([P, P], F16, tag="iota")
            iotac_sb = const_pool.tile([P, 1], F32, tag="iotac")
            iotaf_sb = const_pool.tile([P, P], F32, tag="iotaf")
            ident_sb = const_pool.tile([P, P], F16, tag="ident")
            w1_sb = const_pool.tile([D, D], F16, tag="w1")
            w2_sb = const_pool.tile([D, D], F16, tag="w2")
            b1_sb = const_pool.tile([D, 1], F32, tag="b1")
            b2rep_sb = const_pool.tile([P, D], F32, tag="b2rep")
            nc.sync.dma_start(iota_sb[:], iota_d[:])
            nc.sync.dma_start(iotac_sb[:], iotac_d[:])
            nc.sync.dma_start(iotaf_sb[:], iotaf_d[:])
            nc.sync.dma_start(ident_sb[:], ident_d[:])
            nc.sync.dma_start(w1_sb[:], w1_d[:])
            nc.sync.dma_start(w2_sb[:], w2_d[:])
            nc.sync.dma_start(b1_sb[:], b1_d[:])
            nc.sync.dma_start(b2rep_sb[:], b2rep_d[:])

            rowidx_sb = meta_pool.tile([P, nchunks], I32, tag="rowidx")
            colseg_sb = meta_pool.tile([P, nseg], F32, tag="colseg")
            wseg_sb = meta_pool.tile([P, nseg], F32, tag="wseg")
            selfw_sb = meta_pool.tile([P, max(nfull, 1)], F32, tag="selfw")
            nc.sync.dma_start(rowidx_sb[:], rowidx_d[:])
            nc.sync.dma_start(colseg_sb[:], colseg_d[:])
            nc.sync.dma_start(wseg_sb[:], wseg_d[:])
            nc.sync.dma_start(selfw_sb[:], selfw_d[:])

            t2_own = dram_pool.tile([nblk * P, D], F16, tag="t2own")
            t2_fulls = [
                dram_pool.tile([N, D], F16, tag=f"t2full{r}",
                               addr_space="Shared", name=f"t2full{r}")
                for r in range(repeat)
            ]

            def post_block(layer, b, psum_agg):
                rows = min(P, npc - b * P)
                if layer == 0:
                    agg_sb = w_pool.tile([P, P], F16, tag="agg_sb")
                    nc.vector.tensor_copy(agg_sb[:, :D], psum_agg[:])
                    ptr1 = pmisc_pool.tile([P, P], F16, tag="tr1")
                    nc.tensor.transpose(ptr1[:], agg_sb[:], ident_sb[:])
                    aggT_sb = w_pool.tile([D, P], F16, tag="aggT")
                    nc.scalar.activation(
                        aggT_sb[:], ptr1[:D, :],
                        mybir.ActivationFunctionType.Copy)
                    pz = pmisc_pool.tile([D, P], F32, tag="z")
                    nc.tensor.matmul(out=pz[:], lhsT=w1_sb[:], rhs=aggT_sb[:],
                                     start=True, stop=True)
                    h1T_sb = w_pool.tile([P, P], F16, tag="h1T")
                    nc.scalar.activation(
                        h1T_sb[:D, :], pz[:],
                        mybir.ActivationFunctionType.Relu,
                        bias=b1_sb[:], scale=1.0)
                    pt2 = pmisc_pool.tile([D, P], F32, tag="t2")
                    nc.tensor.matmul(out=pt2[:], lhsT=w2_sb[:],
                                     rhs=h1T_sb[:D, :], start=True, stop=True)
                    t2T_sb = w_pool.tile([P, P], F16, tag="t2T")
                    nc.vector.tensor_copy(t2T_sb[:D, :], pt2[:])
                    ptr2 = pmisc_pool.tile([P, P], F16, tag="tr2")
                    nc.tensor.transpose(ptr2[:], t2T_sb[:], ident_sb[:])
                    t2_sb = w_pool.tile([P, D], F16, tag="t2n")
                    nc.vector.tensor_copy(t2_sb[:], ptr2[:, :D])
                    nc.sync.dma_start(
                        t2_own[b * P:b * P + rows, :], t2_sb[:rows, :])
                else:
                    tmp_sb = w_pool.tile([P, D], F32, tag="tmp")
                    nc.vector.tensor_tensor(
                        out=tmp_sb[:], in0=psum_agg[:], in1=b2rep_sb[:],
                        op=mybir.AluOpType.add)
                    o_sb = w_pool.tile([P, D], F32, tag="osb")
                    nc.scalar.activation(
                        o_sb[:], tmp_sb[:],
                        mybir.ActivationFunctionType.Relu)
                    nc.sync.dma_start(out_d[b * P:(b + 1) * P, :], o_sb[:])

            for rep_i, layer in enumerate([0, 1] * repeat):
                t2_full = t2_fulls[rep_i // 2]
                table = t1 if layer == 0 else t2_full[:]
                own_src = xo_d if layer == 0 else t2_own[:]
                psums = {}
                si = 0
                GBC = 16
                for c0 in range(0, nchunks, GBC):
                    gn = min(GBC, nchunks - c0)
                    gbuf = g_pool.tile([P, GBC * D], F16, tag="gbuf",
                                       name="gbuf")
                    for g in range(gn):
                        nc.gpsimd.indirect_dma_start(
                            out=gbuf[:, g * D:(g + 1) * D],
                            out_offset=None,
                            in_=table,
                            in_offset=bass.IndirectOffsetOnAxis(
                                ap=rowidx_sb[:, c0 + g:c0 + g + 1], axis=0),
                        )
                    gbuf2 = g_pool.tile([P, GBC * D], F16, tag="gbuf2",
                                        name="gbuf2")
                    nc.vector.tensor_copy(gbuf2[:, :gn * D], gbuf[:, :gn * D])
                    for cch in range(c0, c0 + gn):
                      while (mode != 'gonly') and si < nseg and segs[si][0] == cch:
                        _c, b, first, last, _lo, _hi = segs[si]
                        if first:
                            psums[b] = pagg_pool.tile([P, D], F32, tag="agg",
                                                      name="pagg")
                            if b < nfull:
                                own_sb = own_pool.tile([P, D], F16, tag="own")
                                nc.sync.dma_start(
                                    own_sb[:], own_src[b * P:(b + 1) * P, :])
                                sdiag = s_pool.tile([P, P], F16, tag="s")
                                nc.vector.tensor_scalar(
                                    out=sdiag[:],
                                    in0=iota_sb[:],
                                    scalar1=iotac_sb[:],
                                    scalar2=selfw_sb[:, b:b + 1],
                                    op0=mybir.AluOpType.is_equal,
                                    op1=mybir.AluOpType.mult,
                                )
                                nc.tensor.matmul(
                                    out=psums[b][:], lhsT=sdiag[:],
                                    rhs=own_sb[:], start=True, stop=False)
                        GBS = 16
                        if si % GBS == 0:
                            gn2 = min(GBS, nseg - si)
                            s_big = sbig_pool.tile(
                                [P, GBS * P], F16, tag="sbig", name="sbig")
                            vw = s_big[:, :gn2 * P].rearrange(
                                "p (g m) -> p g m", m=P)
                            ia = iotaf_sb[:]
                            ca = colseg_sb[:, si:si + gn2]
                            wa = wseg_sb[:, si:si + gn2]
                            ap_i = bass.AP(ia.tensor, ia.offset,
                                           [list(ia.ap[0]), [0, gn2], [1, P]])
                            ap_c = bass.AP(ca.tensor, ca.offset,
                                           [list(ca.ap[0]), list(ca.ap[1]),
                                            [0, P]])
                            ap_w = bass.AP(wa.tensor, wa.offset,
                                           [list(wa.ap[0]), list(wa.ap[1]),
                                            [0, P]])
                            nc.vector.tensor_tensor(
                                out=vw, in0=ap_i, in1=ap_c,
                                op=mybir.AluOpType.is_equal)
                            nc.vector.tensor_tensor(
                                out=vw, in0=vw, in1=ap_w,
                                op=mybir.AluOpType.mult)
                        s_t = s_big[:, (si % GBS) * P:(si % GBS + 1) * P]
                        nc.tensor.matmul(
                            out=psums[b][:],
                            lhsT=s_t,
                            rhs=gbuf2[:, (cch - c0) * D:(cch - c0 + 1) * D],
                            start=(first and b >= nfull),
                            stop=last,
                        )
                        if last:
                            post_block(layer, b, psums.pop(b))
                        si += 1

                if layer == 0 and not no_coll:
                    nc.gpsimd.collective_compute(
                        "AllGather",
                        mybir.AluOpType.bypass,
                        replica_groups=[list(range(NCORES))],
                        ins=[t2_own[:npc, :]],
                        outs=[t2_full[:]],
                    )

    nc.compile()
    return nc


_CACHE = {}


def _get_program(N, npc, nblk, nfull, nchunks, segs, repeat=1,
                 no_coll=False, mode='full'):
    key = (N, npc, nblk, nfull, nchunks, tuple(segs), repeat, no_coll, mode)
    if key not in _CACHE:
        _CACHE[key] = _build_program(N, npc, nblk, nfull, nchunks, segs,
                                     repeat=repeat, no_coll=no_coll, mode=mode)
    return _CACHE[key]


def _make_inputs(x, W1, b1, W2, b2, pre):
    rowidx, colseg, wseg, selfw, segs, nchunks, npc, nblk, nfull = pre
    t1 = np.asarray(x, np.float32).astype(np.float16)
    common = {
        "t1": t1,
        "iota": np.tile(np.arange(P, dtype=np.float16), (P, 1)),
        "iotac": np.arange(P, dtype=np.float32).reshape(P, 1),
        "iotaf": np.tile(np.arange(P, dtype=np.float32), (P, 1)),
        "ident": np.eye(P, dtype=np.float16),
        "w1": np.asarray(W1, np.float32).astype(np.float16),
        "w2": np.asarray(W2, np.float32).astype(np.float16),
        "b1": np.asarray(b1, np.float32).reshape(D, 1),
        "b2rep": np.tile(np.asarray(b2, np.float32).reshape(1, D), (P, 1)),
    }
    in_maps = []
    for c in range(NCORES):
        xo = np.zeros((nblk * P, D), np.float16)
        xo[:npc] = t1[c * npc:(c + 1) * npc]
        m = dict(common)
        m["x_own"] = xo
        m["rowidx"] = rowidx[c]
        m["colseg"] = colseg[c]
        m["wseg"] = wseg[c]
        m["selfw"] = selfw[c]
        in_maps.append(m)
    return in_maps


def kernel(x, edge_index, edge_weight, batch, W1, b1, W2, b2, **_unused):
    x = np.asarray(x, dtype=np.float32)
    edge_index = np.asarray(edge_index)
    ew = np.asarray(edge_weight, dtype=np.float32)
    N = x.shape[0]
    row = np.asarray(edge_index[0], dtype=np.int64)
    col = np.asarray(edge_index[1], dtype=np.int64)

    pre = _preprocess(row, col, ew, N)
    rowidx, colseg, wseg, selfw, segs, nchunks, npc, nblk, nfull = pre
    nc = _get_program(N, npc, nblk, nfull, nchunks, segs)
    in_maps = _make_inputs(x, W1, b1, W2, b2, pre)

    res = bass_utils.run_bass_kernel_spmd(nc, in_maps, core_ids=list(range(NCORES)))
    out = np.concatenate([res.results[c]["out"][:npc] for c in range(NCORES)],
                         axis=0)
    return out.astype(np.float32)



# revision 8
# speedup vs baseline: 4.8579x; 4.8579x over previous
"""GCN block (2-layer GCNConv + ReLU) on 8 Trainium2 NeuronCores.

Instruction-count-minimal design (this runtime costs ~40us per
instruction, so the per-edge matmul formulation at ~6700 instructions
is dispatch-bound; this program uses ~180).

Per core (owns 6250 target nodes), per layer:
  - Targets are degree-sorted into 49 blocks of 128 (permutation sigma);
    each target's edges (incl. its self-loop) form a vertical run in a
    [128 channel x chunk] grid: channel = rank%128 within its block,
    chunks = consecutive rows of the block's region. Regions have
    uniform height and a single source-half.
  - Sources are split into lo (table row < 32768) / hi halves so the
    int16 dma_gather indices reach all 50176 padded table rows.
  - Per region (<= 8192 slots): one streaming dma_gather (node-major
    rows, 256B each), one broadcast multiply by the per-edge GCN norm,
    and one strided tensor_reduce over the height axis that lands
    per-target sums directly into a staging tile (fp32).
  - lo + hi stagings are added; layer 1 then applies W1 (+b1, relu) and
    W2 feature-major after a single DMA-transpose read, and writes the
    result node-major via one DVE stream-transpose + 3 block-permuted
    DMA writes.  An AllGather shares the (sigma-permuted) t2 table.
  - Layer 2 needs no weights (W2 folded in); its staged result gets
    +b2, relu, and ONE duplicate-free dma_scatter_add that undoes
    sigma into true node order.
"""

import os
import sys

for _p in ("/opt/trn_rl_repo", "/root/.axon_site/_ro/trn_rl_repo"):
    if os.path.isdir(_p) and _p not in sys.path:
        sys.path.insert(0, _p)

import numpy as np

import concourse.bass as bass
import concourse.bacc as bacc
import concourse.mybir as mybir
import concourse.tile as tile
from concourse import bass_utils

F16 = mybir.dt.float16
F32 = mybir.dt.float32
I16 = mybir.dt.int16

NCORES = 8
D = 96
EW = 128                 # padded row width (fp16 -> 256B)
P = 128
HALF = 32768
REG_CAP = 64             # chunks per region (<= 8192 gather indices)
MMW = 512                # matmul moving width (one PSUM bank)


def _pack_half(dh, rank_order, nblk):
    """Pack one source-half's per-target degrees into regions.

    dh: [ntile] degrees for this half in sigma (rank) order, padded to
    nblk*128.  Returns list of regions (b0, nb, h) and total chunks.
    """
    hb = dh.reshape(nblk, P).max(axis=1)          # block heights
    regions = []
    b = 0
    while b < nblk:
        h = int(hb[b])
        if h == 0:
            b += 1
            continue
        nb = 1
        while (b + nb < nblk and hb[b + nb] <= h
               and (nb + 1) * h <= REG_CAP):
            nb += 1
        regions.append((b, nb, h))
        b += nb
    return regions


def _preprocess(row, col, ew, N):
    npc = N // NCORES            # 6250
    nblk = (npc + P - 1) // P    # 49
    ntile = nblk * P             # 6272

    deg = np.bincount(col, weights=ew, minlength=N) + 1.0
    dinv = (1.0 / np.sqrt(deg)).astype(np.float64)
    norm = dinv[row] * ew * dinv[col]
    selfn = dinv * dinv

    core_of = col // npc
    tloc = col - core_of * npc

    # per-core degree (edges + self) of each local target
    d_all = np.zeros((NCORES, ntile), np.int64)
    np.add.at(d_all, (core_of, tloc), 1)
    d_all[:, :npc] += 1          # self-loops
    assert d_all.max() <= REG_CAP, d_all.max()

    # sigma: rank order of targets (degree desc) per core
    rank_order = np.zeros((NCORES, ntile), np.int64)   # slot -> local target
    sigma = np.zeros((NCORES, ntile), np.int64)        # local target -> slot
    for c in range(NCORES):
        o = np.argsort(-d_all[c], kind="stable")
        rank_order[c] = o
        sigma[c, o] = np.arange(ntile)

    # remapped table rows
    r1 = (row // npc) * ntile + (row % npc)            # layer-1 source rows
    r2 = (row // npc) * ntile + sigma[row // npc, row % npc]
    own = np.arange(N)
    r1_self = (own // npc) * ntile + (own % npc)
    r2_self = (own // npc) * ntile + sigma[own // npc, own % npc]

    # per-core, per-layer, per-half edge lists grouped by (slot, rank)
    # layer l edge (src_row_l, slot, norm); half by src_row_l < HALF.
    # Degrees per half (shared structure across layers since src rows of
    # an edge are in the same half?  NOT guaranteed: r1 vs r2 differ ->
    # halves can differ per layer.  Keep per-layer structures.
    pre = {"npc": npc, "nblk": nblk, "ntile": ntile,
           "sigma": sigma, "rank_order": rank_order}

    for L, (rr, rs) in enumerate(((r1, r1_self), (r2, r2_self))):
        # full edge list incl self-loops
        e_src = np.concatenate([rr, rs])
        e_core = np.concatenate([core_of, own // npc])
        e_slot_local = np.concatenate([tloc, own % npc])
        e_slot = sigma[e_core, e_slot_local]
        e_norm = np.concatenate([norm, selfn])
        e_lo = e_src < HALF

        d_h = np.zeros((2, NCORES, ntile), np.int64)
        np.add.at(d_h, ((~e_lo).astype(np.int64), e_core, e_slot), 1)

        # shared region structure: block heights = max over cores
        regions = []
        for h in range(2):
            dh_max = d_h[h].reshape(NCORES, nblk, P).max(axis=(0, 2))
            dh_flat = np.repeat(dh_max, P)
            regions.append(_pack_half(dh_flat, None, nblk))
        tot_chunks = sum(nb * hh for rgs in regions
                         for (_b0, nb, hh) in rgs)

        # chunk base of each block in each half
        chunk_base = np.full((2, nblk), -1, np.int64)
        blk_h = np.zeros((2, nblk), np.int64)
        cb = 0
        reg_list = []   # (half, b0, nb, h, chunk_base)
        for h in range(2):
            for (b0, nb, hh) in regions[h]:
                reg_list.append((h, b0, nb, hh, cb))
                for k in range(nb):
                    chunk_base[h, b0 + k] = cb + k * hh
                    blk_h[h, b0 + k] = hh
                cb += nb * hh
        assert cb == tot_chunks

        # slot position of every edge: rank within (core, half, slot)
        order = np.lexsort((e_norm, e_src, e_slot, e_core, ~e_lo))
        es, ec, eslot, enorm, elo = (e_src[order], e_core[order],
                                     e_slot[order], e_norm[order],
                                     e_lo[order])
        ehalf = (~elo).astype(np.int64)
        key = ((ehalf * NCORES + ec) * ntile + eslot)
        first = np.ones(len(key), bool)
        first[1:] = key[1:] != key[:-1]
        idx_in_run = np.arange(len(key)) - np.maximum.accumulate(
            np.where(first, np.arange(len(key)), 0))
        # chunk of edge = chunk_base[half, block] + rank; channel = slot%128
        eblk = eslot // P
        echunk = chunk_base[ehalf, eblk] + idx_in_run
        assert (idx_in_run < blk_h[ehalf, eblk]).all()
        echan = eslot % P

        # gather idx / norm arrays
        gidx = np.zeros((NCORES, P, tot_chunks * 8), np.int16)
        gnorm = np.zeros((NCORES, P, tot_chunks), np.float16)
        src_rel = np.where(elo, es, es - HALF).astype(np.int64)
        pos = echunk * P + echan
        # wrap: position i -> [16k + i%16, i//16]
        prow = (pos % 16).astype(np.int64)
        pcol = (pos // 16).astype(np.int64)
        for k in range(8):
            gidx[ec, 16 * k + prow, pcol] = src_rel
        gnorm[ec, echan, echunk] = enorm

        pre[f"gidx{L}"] = gidx
        pre[f"gnorm{L}"] = gnorm
        pre[f"regs{L}"] = reg_list
        pre[f"chunks{L}"] = tot_chunks

    # scatter indices (undo sigma; pads -> distinct unused rows)
    sidx = np.zeros((NCORES, P, ntile // 16), np.int16)
    for c in range(NCORES):
        tgt = np.where(rank_order[c] < ntile, rank_order[c], 0)
        # slots holding real targets scatter there; pad slots (target id
        # >= npc would collide) -> unused rows npc..ntile-1, distinct.
        pad_rows = np.arange(npc, ntile)
        is_pad = rank_order[c] >= npc
        tgt = rank_order[c].copy()
        tgt[is_pad] = pad_rows[:is_pad.sum()]
        pos = np.arange(ntile)
        for k in range(8):
            sidx[c, 16 * k + pos % 16, pos // 16] = tgt
    pre["sidx"] = sidx
    return pre


def _build_program(pre, repeat=1, no_coll=False):
    npc, nblk, ntile = pre["npc"], pre["nblk"], pre["ntile"]
    NROWS = NCORES * ntile
    ch0, ch1 = pre["chunks0"], pre["chunks1"]

    nc = bacc.Bacc("TRN2", target_bir_lowering=False, debug=False,
                   enable_asserts=False, num_devices=NCORES)

    xpad_d = nc.dram_tensor("xpad", [NROWS, EW], F16, kind="ExternalInput").ap()
    gidx0_d = nc.dram_tensor("gidx0", [P, ch0 * 8], I16, kind="ExternalInput").ap()
    gidx1_d = nc.dram_tensor("gidx1", [P, ch1 * 8], I16, kind="ExternalInput").ap()
    gnorm0_d = nc.dram_tensor("gnorm0", [P, ch0], F16, kind="ExternalInput").ap()
    gnorm1_d = nc.dram_tensor("gnorm1", [P, ch1], F16, kind="ExternalInput").ap()
    sidx_d = nc.dram_tensor("sidx", [P, ntile // 16], I16, kind="ExternalInput").ap()
    w1_d = nc.dram_tensor("w1", [D, D], F16, kind="ExternalInput").ap()
    w2_d = nc.dram_tensor("w2", [D, D], F16, kind="ExternalInput").ap()
    b1_d = nc.dram_tensor("b1", [D, 1], F32, kind="ExternalInput").ap()
    b2rep_d = nc.dram_tensor("b2rep", [P, EW], F32, kind="ExternalInput").ap()
    out_d = nc.dram_tensor("out", [ntile, D], F32, kind="ExternalOutput").ap()

    NSL = 13  # ceil(6272/512) matmul slices

    with tile.TileContext(nc) as tc:
        with (
            tc.tile_pool(name="const", bufs=1) as cpool,
            tc.tile_pool(name="gath", bufs=2) as gpool,
            tc.tile_pool(name="stage", bufs=1) as spool,
            tc.tile_pool(name="work", bufs=1) as wpool,
            tc.tile_pool(name="psum", bufs=4, space="PSUM") as ppool,
            tc.tile_pool(name="dram", bufs=1, space="DRAM") as dpool,
        ):
            # ---- persistent loads (outside the repeated body)
            chmax = max(ch0, ch1)
            gidx_sb = cpool.tile([P, chmax * 8], I16, tag="gidx")
            gnorm_sb = cpool.tile([P, chmax], F16, tag="gnorm")
            sidx_sb = cpool.tile([P, ntile // 16], I16, tag="sidx")
            w1_sb = cpool.tile([D, D], F16, tag="w1")
            w2_sb = cpool.tile([D, D], F16, tag="w2")
            b1_sb = cpool.tile([D, 1], F32, tag="b1")
            b2rep_sb = cpool.tile([P, EW], F32, tag="b2rep")
            zero_sb = cpool.tile([P, nblk * EW], F16, tag="zero")
            for sb, dr in ((sidx_sb, sidx_d), (w1_sb, w1_d), (w2_sb, w2_d),
                           (b1_sb, b1_d), (b2rep_sb, b2rep_d)):
                nc.sync.dma_start(sb[:], dr[:])
            nc.vector.memset(zero_sb[:], 0.0)

            t2_own = dpool.tile([ntile, EW], F16, tag="t2own")
            # zero pad cols once (perm writes only cols :96)
            nc.sync.dma_start(
                t2_own[:].rearrange("(b p) e -> p b e", p=P),
                zero_sb[:].rearrange("p (b e) -> p b e", e=EW))
            t2_fulls = [
                dpool.tile([NROWS, EW], F16, tag=f"t2full{r}",
                           addr_space="Shared", name=f"t2full{r}")
                for r in range(repeat)
            ]
            agg_perm = dpool.tile([ntile, EW], F16, tag="aggperm")
            outf16 = dpool.tile([ntile, EW], F16, tag="outf16")

            def aggregate(L, table, gidx_d_l, gnorm_d_l, ch_l):
                """gather+norm+reduce all regions; returns fp32 stagings."""
                nc.sync.dma_start(gidx_sb[:, :ch_l * 8], gidx_d_l[:])
                nc.sync.dma_start(gnorm_sb[:, :ch_l], gnorm_d_l[:])
                st = [spool.tile([P, nblk * EW], F32, tag=f"stg{h}",
                                 name=f"stg{h}")
                      for h in range(2)]
                nc.vector.memset(st[0][:], 0.0)
                nc.vector.memset(st[1][:], 0.0)
                for (h, b0, nb, hh, cb) in pre[f"regs{L}"]:
                    nch = nb * hh
                    ni = nch * P
                    g = gpool.tile([P, REG_CAP * EW], F16, tag="reg",
                                   name="reg")
                    src = table if h == 0 else table[HALF:, :]
                    nc.gpsimd.dma_gather(
                        out_ap=g[:, :nch * EW].rearrange(
                            "p (g e) -> p g e", e=EW),
                        in_ap=src,
                        idxs_ap=gidx_sb[:, cb * 8:(cb + nch) * 8],
                        num_idxs=ni,
                        num_idxs_reg=ni,
                        elem_size=EW,
                        single_packet=False,
                    )
                    gv = g[:, :nch * EW].rearrange("p (g e) -> p g e", e=EW)
                    na = gnorm_sb[:, cb:cb + nch]
                    nb_ap = bass.AP(na.tensor, na.offset,
                                    [list(na.ap[0]), list(na.ap[1]), [0, EW]])
                    nc.vector.tensor_tensor(out=gv, in0=gv, in1=nb_ap,
                                            op=mybir.AluOpType.mult)
                    pa = g[:]
                    rin = bass.AP(pa.tensor, pa.offset,
                                  [list(pa.ap[0]), [hh * EW, nb], [1, EW],
                                   [EW, hh]])
                    sa = st[h][:, b0 * EW:(b0 + nb) * EW]
                    rout = bass.AP(sa.tensor, sa.offset,
                                   [list(sa.ap[0]), [EW, nb], [1, EW]])
                    nc.vector.tensor_reduce(out=rout, in_=rin,
                                            axis=mybir.AxisListType.X,
                                            op=mybir.AluOpType.add)
                return st

            for rep in range(repeat):
                t2_full = t2_fulls[rep]

                # ================= layer 1 =================
                st = aggregate(0, xpad_d, gidx0_d, gnorm0_d, ch0)
                agg_sb = wpool.tile([P, nblk * EW], F16, tag="agg16")
                nc.vector.tensor_tensor(out=agg_sb[:], in0=st[0][:],
                                        in1=st[1][:], op=mybir.AluOpType.add)
                nc.sync.dma_start(
                    agg_perm[:].rearrange("(b p) e -> p b e", p=P),
                    agg_sb[:].rearrange("p (b e) -> p b e", e=EW))
                aggT_sb = wpool.tile([P, ntile], F16, tag="big1")
                nc.sync.dma_start(aggT_sb[:], agg_perm[:], transpose=True)

                h1T_sb = wpool.tile([D, ntile], F16, tag="h1T")
                for s in range(NSL):
                    c0, c1 = s * MMW, min((s + 1) * MMW, ntile)
                    pz = ppool.tile([D, MMW], F32, tag="pz", name="pz")
                    nc.tensor.matmul(out=pz[:, :c1 - c0], lhsT=w1_sb[:],
                                     rhs=aggT_sb[:D, c0:c1],
                                     start=True, stop=True)
                    nc.scalar.activation(
                        h1T_sb[:, c0:c1], pz[:, :c1 - c0],
                        mybir.ActivationFunctionType.Relu,
                        bias=b1_sb[:], scale=1.0)
                t2T_sb = wpool.tile([P, ntile], F16, tag="agg16")
                for s in range(NSL):
                    c0, c1 = s * MMW, min((s + 1) * MMW, ntile)
                    pz = ppool.tile([D, MMW], F32, tag="pz2", name="pz2")
                    nc.tensor.matmul(out=pz[:, :c1 - c0], lhsT=w2_sb[:],
                                     rhs=h1T_sb[:, c0:c1],
                                     start=True, stop=True)
                    nc.scalar.activation(
                        t2T_sb[:D, c0:c1], pz[:, :c1 - c0],
                        mybir.ActivationFunctionType.Copy)

                # node-major write: stream-transpose + 3 perm DMAs
                tt_sb = wpool.tile([P, ntile], F16, tag="big1")
                nc.vector.transpose(tt_sb[:D, :], t2T_sb[:D, :])
                NB = ntile // 32
                for bi in range(3):
                    srcap = tt_sb[32 * bi:32 * (bi + 1), :].rearrange(
                        "a (bj b) -> a bj b", b=32)
                    da = t2_own[:]
                    dst = bass.AP(da.tensor, da.offset + 32 * bi,
                                  [[EW, 32], [32 * EW, NB], [1, 32]])
                    nc.sync.dma_start(dst, srcap)

                if not no_coll:
                    nc.gpsimd.collective_compute(
                        "AllGather",
                        mybir.AluOpType.bypass,
                        replica_groups=[list(range(NCORES))],
                        ins=[t2_own[:]],
                        outs=[t2_full[:]],
                    )

                # ================= layer 2 =================
                st = aggregate(1, t2_full[:], gidx1_d, gnorm1_d, ch1)
                nc.vector.tensor_tensor(out=st[0][:], in0=st[0][:],
                                        in1=st[1][:], op=mybir.AluOpType.add)
                b2b = b2rep_sb[:]
                b2_ap = bass.AP(b2b.tensor, b2b.offset,
                                [list(b2b.ap[0]), [0, nblk], [1, EW]])
                nc.vector.tensor_tensor(
                    out=st[0][:].rearrange("p (b e) -> p b e", e=EW),
                    in0=st[0][:].rearrange("p (b e) -> p b e", e=EW),
                    in1=b2_ap, op=mybir.AluOpType.add)
                stg16 = wpool.tile([P, nblk * EW], F16, tag="agg16")
                nc.scalar.activation(stg16[:], st[0][:],
                                     mybir.ActivationFunctionType.Relu)
                # zero the scatter target, then undo sigma in one scatter
                nc.sync.dma_start(
                    outf16[:].rearrange("(b p) e -> p b e", p=P),
                    zero_sb[:].rearrange("p (b e) -> p b e", e=EW))
                nc.gpsimd.dma_scatter_add(
                    out_ap=outf16[:],
                    in_ap=stg16[:].rearrange("p (g e) -> p g e", e=EW),
                    idxs_ap=sidx_sb[:],
                    num_idxs=ntile,
                    num_idxs_reg=ntile,
                    elem_size=EW,
                    single_packet=False,
                )
                nc.gpsimd.dma_start(
                    out_d[:].rearrange("(b p) e -> p b e", p=P),
                    outf16[:].rearrange("(b p) e -> p b e", p=P)[:, :, :D])

    nc.compile()
    return nc


_CACHE = {}


def _get_program(pre, repeat=1, no_coll=False):
    key = (pre["chunks0"], pre["chunks1"], tuple(pre["regs0"]),
           tuple(pre["regs1"]), repeat, no_coll)
    if key not in _CACHE:
        _CACHE[key] = _build_program(pre, repeat=repeat, no_coll=no_coll)
    return _CACHE[key]


def _make_inputs(x, W1, b1, W2, b2, pre):
    npc, ntile = pre["npc"], pre["ntile"]
    N = NCORES * npc
    xpad = np.zeros((NCORES * ntile, EW), np.float16)
    xv = np.asarray(x, np.float32).astype(np.float16)
    for c in range(NCORES):
        xpad[c * ntile:c * ntile + npc, :D] = xv[c * npc:(c + 1) * npc]
    b2rep = np.zeros((P, EW), np.float32)
    b2rep[:, :D] = np.asarray(b2, np.float32)[None, :]
    common = {
        "xpad": xpad,
        "w1": np.asarray(W1, np.float32).astype(np.float16),
        "w2": np.asarray(W2, np.float32).astype(np.float16),
        "b1": np.asarray(b1, np.float32).reshape(D, 1),
        "b2rep": b2rep,
    }
    in_maps = []
    for c in range(NCORES):
        m = dict(common)
        m["gidx0"] = pre["gidx0"][c]
        m["gidx1"] = pre["gidx1"][c]
        m["gnorm0"] = pre["gnorm0"][c]
        m["gnorm1"] = pre["gnorm1"][c]
        m["sidx"] = pre["sidx"][c]
        in_maps.append(m)
    return in_maps


def kernel(x, edge_index, edge_weight, batch, W1, b1, W2, b2, **_unused):
    x = np.asarray(x, dtype=np.float32)
    edge_index = np.asarray(edge_index)
    ew = np.asarray(edge_weight, dtype=np.float64)
    N = x.shape[0]
    row = np.asarray(edge_index[0], dtype=np.int64)
    col = np.asarray(edge_index[1], dtype=np.int64)

    pre = _preprocess(row, col, ew, N)
    nc = _get_program(pre)
    in_maps = _make_inputs(x, W1, b1, W2, b2, pre)
    res = bass_utils.run_bass_kernel_spmd(nc, in_maps,
                                          core_ids=list(range(NCORES)))
    npc, ntile = pre["npc"], pre["ntile"]
    out = np.concatenate([res.results[c]["out"][:npc] for c in range(NCORES)],
                         axis=0)
    return out.astype(np.float32)


# revision 9
# speedup vs baseline: 11.3989x; 2.3465x over previous
"""GCN block (2-layer GCNConv + ReLU) on 8 Trainium2 NeuronCores.

Instruction-count-minimal design (this runtime costs ~40us per
instruction, so the per-edge matmul formulation at ~6700 instructions
is dispatch-bound; this program uses ~180).

Per core (owns 6250 target nodes), per layer:
  - Targets are degree-sorted into 49 blocks of 128 (permutation sigma);
    each target's edges (incl. its self-loop) form a vertical run in a
    [128 channel x chunk] grid: channel = rank%128 within its block,
    chunks = consecutive rows of the block's region. Regions have
    uniform height and a single source-half.
  - Sources are split into lo (table row < 32768) / hi halves so the
    int16 dma_gather indices reach all 50176 padded table rows.
  - Per region (<= 8192 slots): one streaming dma_gather (node-major
    rows, 256B each), one broadcast multiply by the per-edge GCN norm,
    and one strided tensor_reduce over the height axis that lands
    per-target sums directly into a staging tile (fp32).
  - lo + hi stagings are added; layer 1 then applies W1 (+b1, relu) and
    W2 feature-major after a single DMA-transpose read, and writes the
    result node-major via one DVE stream-transpose + 3 block-permuted
    DMA writes.  An AllGather shares the (sigma-permuted) t2 table.
  - Layer 2 needs no weights (W2 folded in); its staged result gets
    +b2, relu, and ONE duplicate-free dma_scatter_add that undoes
    sigma into true node order.
"""

import os
import sys

for _p in ("/opt/trn_rl_repo", "/root/.axon_site/_ro/trn_rl_repo"):
    if os.path.isdir(_p) and _p not in sys.path:
        sys.path.insert(0, _p)

import numpy as np

import concourse.bass as bass
import concourse.bacc as bacc
import concourse.mybir as mybir
import concourse.tile as tile
from concourse import bass_utils

F16 = mybir.dt.float16
F32 = mybir.dt.float32
I16 = mybir.dt.int16

NCORES = 8
D = 96
EW = 128                 # padded row width (fp16 -> 256B)
P = 128
HALF = 32768
REG_CAP = 64             # chunks per region (<= 8192 gather indices)
MMW = 512                # matmul moving width (one PSUM bank)


def _pack_half(dh, rank_order, nblk):
    """Pack one source-half's per-target degrees into regions.

    dh: [ntile] degrees for this half in sigma (rank) order, padded to
    nblk*128.  Returns list of regions (b0, nb, h) and total chunks.
    """
    hb = dh.reshape(nblk, P).max(axis=1)          # block heights
    regions = []
    b = 0
    while b < nblk:
        h = int(hb[b])
        if h == 0:
            b += 1
            continue
        nb = 1
        while (b + nb < nblk and hb[b + nb] <= h
               and (nb + 1) * h <= REG_CAP):
            nb += 1
        regions.append((b, nb, h))
        b += nb
    return regions


def _preprocess(row, col, ew, N):
    npc = N // NCORES            # 6250
    nblk = (npc + P - 1) // P    # 49
    ntile = nblk * P             # 6272

    deg = np.bincount(col, weights=ew, minlength=N) + 1.0
    dinv = (1.0 / np.sqrt(deg)).astype(np.float64)
    norm = dinv[row] * ew * dinv[col]
    selfn = dinv * dinv

    core_of = col // npc
    tloc = col - core_of * npc

    # per-core degree (edges + self) of each local target
    d_all = np.zeros((NCORES, ntile), np.int64)
    np.add.at(d_all, (core_of, tloc), 1)
    d_all[:, :npc] += 1          # self-loops
    assert d_all.max() <= REG_CAP, d_all.max()

    # sigma: rank order of targets (degree desc) per core
    rank_order = np.zeros((NCORES, ntile), np.int64)   # slot -> local target
    sigma = np.zeros((NCORES, ntile), np.int64)        # local target -> slot
    for c in range(NCORES):
        o = np.argsort(-d_all[c], kind="stable")
        rank_order[c] = o
        sigma[c, o] = np.arange(ntile)

    # remapped table rows
    r1 = (row // npc) * ntile + (row % npc)            # layer-1 source rows
    r2 = (row // npc) * ntile + sigma[row // npc, row % npc]
    own = np.arange(N)
    r1_self = (own // npc) * ntile + (own % npc)
    r2_self = (own // npc) * ntile + sigma[own // npc, own % npc]

    # per-core, per-layer, per-half edge lists grouped by (slot, rank)
    # layer l edge (src_row_l, slot, norm); half by src_row_l < HALF.
    # Degrees per half (shared structure across layers since src rows of
    # an edge are in the same half?  NOT guaranteed: r1 vs r2 differ ->
    # halves can differ per layer.  Keep per-layer structures.
    pre = {"npc": npc, "nblk": nblk, "ntile": ntile,
           "sigma": sigma, "rank_order": rank_order}

    for L, (rr, rs) in enumerate(((r1, r1_self), (r2, r2_self))):
        # full edge list incl self-loops
        e_src = np.concatenate([rr, rs])
        e_core = np.concatenate([core_of, own // npc])
        e_slot_local = np.concatenate([tloc, own % npc])
        e_slot = sigma[e_core, e_slot_local]
        e_norm = np.concatenate([norm, selfn])
        e_lo = e_src < HALF

        d_h = np.zeros((2, NCORES, ntile), np.int64)
        np.add.at(d_h, ((~e_lo).astype(np.int64), e_core, e_slot), 1)

        # shared region structure: block heights = max over cores
        regions = []
        for h in range(2):
            dh_max = d_h[h].reshape(NCORES, nblk, P).max(axis=(0, 2))
            dh_flat = np.repeat(dh_max, P)
            regions.append(_pack_half(dh_flat, None, nblk))
        tot_chunks = sum(nb * hh for rgs in regions
                         for (_b0, nb, hh) in rgs)

        # chunk base of each block in each half
        chunk_base = np.full((2, nblk), -1, np.int64)
        blk_h = np.zeros((2, nblk), np.int64)
        cb = 0
        reg_list = []   # (half, b0, nb, h, chunk_base)
        for h in range(2):
            for (b0, nb, hh) in regions[h]:
                reg_list.append((h, b0, nb, hh, cb))
                for k in range(nb):
                    chunk_base[h, b0 + k] = cb + k * hh
                    blk_h[h, b0 + k] = hh
                cb += nb * hh
        assert cb == tot_chunks

        # slot position of every edge: rank within (core, half, slot)
        order = np.lexsort((e_norm, e_src, e_slot, e_core, ~e_lo))
        es, ec, eslot, enorm, elo = (e_src[order], e_core[order],
                                     e_slot[order], e_norm[order],
                                     e_lo[order])
        ehalf = (~elo).astype(np.int64)
        key = ((ehalf * NCORES + ec) * ntile + eslot)
        first = np.ones(len(key), bool)
        first[1:] = key[1:] != key[:-1]
        idx_in_run = np.arange(len(key)) - np.maximum.accumulate(
            np.where(first, np.arange(len(key)), 0))
        # chunk of edge = chunk_base[half, block] + rank; channel = slot%128
        eblk = eslot // P
        echunk = chunk_base[ehalf, eblk] + idx_in_run
        assert (idx_in_run < blk_h[ehalf, eblk]).all()
        echan = eslot % P

        # gather idx / norm arrays
        gidx = np.zeros((NCORES, P, tot_chunks * 8), np.int16)
        gnorm = np.zeros((NCORES, P, tot_chunks), np.float16)
        src_rel = np.where(elo, es, es - HALF).astype(np.int64)
        pos = echunk * P + echan
        # wrap: position i -> [16k + i%16, i//16]
        prow = (pos % 16).astype(np.int64)
        pcol = (pos // 16).astype(np.int64)
        for k in range(8):
            gidx[ec, 16 * k + prow, pcol] = src_rel
        gnorm[ec, echan, echunk] = enorm

        pre[f"gidx{L}"] = gidx
        pre[f"gnorm{L}"] = gnorm
        pre[f"regs{L}"] = reg_list
        pre[f"chunks{L}"] = tot_chunks

    # scatter indices (undo sigma; pads -> distinct unused rows)
    sidx = np.zeros((NCORES, P, ntile // 16), np.int16)
    for c in range(NCORES):
        tgt = np.where(rank_order[c] < ntile, rank_order[c], 0)
        # slots holding real targets scatter there; pad slots (target id
        # >= npc would collide) -> unused rows npc..ntile-1, distinct.
        pad_rows = np.arange(npc, ntile)
        is_pad = rank_order[c] >= npc
        tgt = rank_order[c].copy()
        tgt[is_pad] = pad_rows[:is_pad.sum()]
        pos = np.arange(ntile)
        for k in range(8):
            sidx[c, 16 * k + pos % 16, pos // 16] = tgt
    pre["sidx"] = sidx
    return pre


def _build_program(pre, repeat=1, no_coll=False):
    npc, nblk, ntile = pre["npc"], pre["nblk"], pre["ntile"]
    NROWS = NCORES * ntile
    ch0, ch1 = pre["chunks0"], pre["chunks1"]

    nc = bacc.Bacc("TRN2", target_bir_lowering=False, debug=False,
                   enable_asserts=False, num_devices=NCORES,
                   num_swdge_queues=4)

    xpad_d = nc.dram_tensor("xpad", [NROWS, EW], F16, kind="ExternalInput").ap()
    gidx0_d = nc.dram_tensor("gidx0", [P, ch0 * 8], I16, kind="ExternalInput").ap()
    gidx1_d = nc.dram_tensor("gidx1", [P, ch1 * 8], I16, kind="ExternalInput").ap()
    gnorm0_d = nc.dram_tensor("gnorm0", [P, ch0], F16, kind="ExternalInput").ap()
    gnorm1_d = nc.dram_tensor("gnorm1", [P, ch1], F16, kind="ExternalInput").ap()
    sidx_d = nc.dram_tensor("sidx", [P, ntile // 16], I16, kind="ExternalInput").ap()
    w1_d = nc.dram_tensor("w1", [D, D], F16, kind="ExternalInput").ap()
    w2_d = nc.dram_tensor("w2", [D, D], F16, kind="ExternalInput").ap()
    b1_d = nc.dram_tensor("b1", [D, 1], F32, kind="ExternalInput").ap()
    b2rep_d = nc.dram_tensor("b2rep", [P, EW], F32, kind="ExternalInput").ap()
    out_d = nc.dram_tensor("out", [ntile, D], F32, kind="ExternalOutput").ap()

    NSL = 13  # ceil(6272/512) matmul slices

    with tile.TileContext(nc) as tc:
        with (
            tc.tile_pool(name="const", bufs=1) as cpool,
            tc.tile_pool(name="gath", bufs=2) as gpool,
            tc.tile_pool(name="stage", bufs=1) as spool,
            tc.tile_pool(name="work", bufs=1) as wpool,
            tc.tile_pool(name="psum", bufs=4, space="PSUM") as ppool,
            tc.tile_pool(name="dram", bufs=1, space="DRAM") as dpool,
        ):
            # ---- persistent loads (outside the repeated body)
            chmax = max(ch0, ch1)
            gidx_sb = cpool.tile([P, chmax * 8], I16, tag="gidx")
            gnorm_sb = cpool.tile([P, chmax], F16, tag="gnorm")
            sidx_sb = cpool.tile([P, ntile // 16], I16, tag="sidx")
            w1_sb = cpool.tile([D, D], F16, tag="w1")
            w2_sb = cpool.tile([D, D], F16, tag="w2")
            b1_sb = cpool.tile([D, 1], F32, tag="b1")
            b2rep_sb = cpool.tile([P, EW], F32, tag="b2rep")
            zero_sb = cpool.tile([P, nblk * EW], F16, tag="zero")
            for sb, dr in ((sidx_sb, sidx_d), (w1_sb, w1_d), (w2_sb, w2_d),
                           (b1_sb, b1_d), (b2rep_sb, b2rep_d)):
                nc.sync.dma_start(sb[:], dr[:])
            nc.vector.memset(zero_sb[:], 0.0)

            t2_own = dpool.tile([ntile, EW], F16, tag="t2own")
            # zero pad cols once (perm writes only cols :96)
            nc.sync.dma_start(
                t2_own[:].rearrange("(b p) e -> p b e", p=P),
                zero_sb[:].rearrange("p (b e) -> p b e", e=EW))
            t2_fulls = [
                dpool.tile([NROWS, EW], F16, tag=f"t2full{r}",
                           addr_space="Shared", name=f"t2full{r}")
                for r in range(repeat)
            ]
            agg_perm = dpool.tile([ntile, EW], F16, tag="aggperm")
            outf16 = dpool.tile([ntile, EW], F16, tag="outf16")

            def aggregate(L, table, gidx_d_l, gnorm_d_l, ch_l):
                """gather+norm+reduce all regions; returns fp32 stagings."""
                nc.sync.dma_start(gidx_sb[:, :ch_l * 8], gidx_d_l[:])
                nc.sync.dma_start(gnorm_sb[:, :ch_l], gnorm_d_l[:])
                st = [spool.tile([P, nblk * EW], F32, tag=f"stg{h}",
                                 name=f"stg{h}")
                      for h in range(2)]
                nc.vector.memset(st[0][:], 0.0)
                nc.vector.memset(st[1][:], 0.0)
                for ri, (h, b0, nb, hh, cb) in enumerate(pre[f"regs{L}"]):
                    nch = nb * hh
                    ni = nch * P
                    g = gpool.tile([P, REG_CAP * EW], F16, tag="reg",
                                   name="reg")
                    src = table if h == 0 else table[HALF:, :]
                    nc.gpsimd.dma_gather(
                        out_ap=g[:, :nch * EW].rearrange(
                            "p (g e) -> p g e", e=EW),
                        in_ap=src,
                        idxs_ap=gidx_sb[:, cb * 8:(cb + nch) * 8],
                        num_idxs=ni,
                        num_idxs_reg=ni,
                        elem_size=EW,
                        single_packet=False,
                        queue_num=ri % 4,
                    )
                    gv = g[:, :nch * EW].rearrange("p (g e) -> p g e", e=EW)
                    na = gnorm_sb[:, cb:cb + nch]
                    nb_ap = bass.AP(na.tensor, na.offset,
                                    [list(na.ap[0]), list(na.ap[1]), [0, EW]])
                    nc.vector.tensor_tensor(out=gv, in0=gv, in1=nb_ap,
                                            op=mybir.AluOpType.mult)
                    pa = g[:]
                    rin = bass.AP(pa.tensor, pa.offset,
                                  [list(pa.ap[0]), [hh * EW, nb], [1, EW],
                                   [EW, hh]])
                    sa = st[h][:, b0 * EW:(b0 + nb) * EW]
                    rout = bass.AP(sa.tensor, sa.offset,
                                   [list(sa.ap[0]), [EW, nb], [1, EW]])
                    nc.vector.tensor_reduce(out=rout, in_=rin,
                                            axis=mybir.AxisListType.X,
                                            op=mybir.AluOpType.add)
                return st

            for rep in range(repeat):
                t2_full = t2_fulls[rep]

                # ================= layer 1 =================
                st = aggregate(0, xpad_d, gidx0_d, gnorm0_d, ch0)
                agg_sb = wpool.tile([P, nblk * EW], F16, tag="agg16")
                nc.vector.tensor_tensor(out=agg_sb[:], in0=st[0][:],
                                        in1=st[1][:], op=mybir.AluOpType.add)
                nc.sync.dma_start(
                    agg_perm[:].rearrange("(b p) e -> p b e", p=P),
                    agg_sb[:].rearrange("p (b e) -> p b e", e=EW))
                aggT_sb = wpool.tile([P, ntile], F16, tag="big1")
                nc.sync.dma_start(aggT_sb[:], agg_perm[:], transpose=True)

                h1T_sb = wpool.tile([D, ntile], F16, tag="h1T")
                for s in range(NSL):
                    c0, c1 = s * MMW, min((s + 1) * MMW, ntile)
                    pz = ppool.tile([D, MMW], F32, tag="pz", name="pz")
                    nc.tensor.matmul(out=pz[:, :c1 - c0], lhsT=w1_sb[:],
                                     rhs=aggT_sb[:D, c0:c1],
                                     start=True, stop=True)
                    nc.scalar.activation(
                        h1T_sb[:, c0:c1], pz[:, :c1 - c0],
                        mybir.ActivationFunctionType.Relu,
                        bias=b1_sb[:], scale=1.0)
                t2T_sb = wpool.tile([P, ntile], F16, tag="agg16")
                for s in range(NSL):
                    c0, c1 = s * MMW, min((s + 1) * MMW, ntile)
                    pz = ppool.tile([D, MMW], F32, tag="pz2", name="pz2")
                    nc.tensor.matmul(out=pz[:, :c1 - c0], lhsT=w2_sb[:],
                                     rhs=h1T_sb[:, c0:c1],
                                     start=True, stop=True)
                    nc.scalar.activation(
                        t2T_sb[:D, c0:c1], pz[:, :c1 - c0],
                        mybir.ActivationFunctionType.Copy)

                # node-major write: stream-transpose + 3 perm DMAs
                tt_sb = wpool.tile([P, ntile], F16, tag="big1")
                nc.vector.transpose(tt_sb[:D, :], t2T_sb[:D, :])
                NB = ntile // 32
                for bi in range(3):
                    srcap = tt_sb[32 * bi:32 * (bi + 1), :].rearrange(
                        "a (bj b) -> a bj b", b=32)
                    da = t2_own[:]
                    dst = bass.AP(da.tensor, da.offset + 32 * bi,
                                  [[EW, 32], [32 * EW, NB], [1, 32]])
                    nc.sync.dma_start(dst, srcap)

                if not no_coll:
                    nc.gpsimd.collective_compute(
                        "AllGather",
                        mybir.AluOpType.bypass,
                        replica_groups=[list(range(NCORES))],
                        ins=[t2_own[:]],
                        outs=[t2_full[:]],
                    )

                # ================= layer 2 =================
                st = aggregate(1, t2_full[:], gidx1_d, gnorm1_d, ch1)
                nc.vector.tensor_tensor(out=st[0][:], in0=st[0][:],
                                        in1=st[1][:], op=mybir.AluOpType.add)
                b2b = b2rep_sb[:]
                b2_ap = bass.AP(b2b.tensor, b2b.offset,
                                [list(b2b.ap[0]), [0, nblk], [1, EW]])
                nc.vector.tensor_tensor(
                    out=st[0][:].rearrange("p (b e) -> p b e", e=EW),
                    in0=st[0][:].rearrange("p (b e) -> p b e", e=EW),
                    in1=b2_ap, op=mybir.AluOpType.add)
                stg16 = wpool.tile([P, nblk * EW], F16, tag="agg16")
                nc.scalar.activation(stg16[:], st[0][:],
                                     mybir.ActivationFunctionType.Relu)
                # zero the scatter target, then undo sigma in one scatter
                nc.sync.dma_start(
                    outf16[:].rearrange("(b p) e -> p b e", p=P),
                    zero_sb[:].rearrange("p (b e) -> p b e", e=EW))
                nc.gpsimd.dma_scatter_add(
                    out_ap=outf16[:],
                    in_ap=stg16[:].rearrange("p (g e) -> p g e", e=EW),
                    idxs_ap=sidx_sb[:],
                    num_idxs=ntile,
                    num_idxs_reg=ntile,
                    elem_size=EW,
                    single_packet=False,
                )
                nc.gpsimd.dma_start(
                    out_d[:].rearrange("(b p) e -> p b e", p=P),
                    outf16[:].rearrange("(b p) e -> p b e", p=P)[:, :, :D])

    nc.compile()
    return nc


_CACHE = {}


def _get_program(pre, repeat=1, no_coll=False):
    key = (pre["chunks0"], pre["chunks1"], tuple(pre["regs0"]),
           tuple(pre["regs1"]), repeat, no_coll)
    if key not in _CACHE:
        _CACHE[key] = _build_program(pre, repeat=repeat, no_coll=no_coll)
    return _CACHE[key]


def _make_inputs(x, W1, b1, W2, b2, pre):
    npc, ntile = pre["npc"], pre["ntile"]
    N = NCORES * npc
    xpad = np.zeros((NCORES * ntile, EW), np.float16)
    xv = np.asarray(x, np.float32).astype(np.float16)
    for c in range(NCORES):
        xpad[c * ntile:c * ntile + npc, :D] = xv[c * npc:(c + 1) * npc]
    b2rep = np.zeros((P, EW), np.float32)
    b2rep[:, :D] = np.asarray(b2, np.float32)[None, :]
    common = {
        "xpad": xpad,
        "w1": np.asarray(W1, np.float32).astype(np.float16),
        "w2": np.asarray(W2, np.float32).astype(np.float16),
        "b1": np.asarray(b1, np.float32).reshape(D, 1),
        "b2rep": b2rep,
    }
    in_maps = []
    for c in range(NCORES):
        m = dict(common)
        m["gidx0"] = pre["gidx0"][c]
        m["gidx1"] = pre["gidx1"][c]
        m["gnorm0"] = pre["gnorm0"][c]
        m["gnorm1"] = pre["gnorm1"][c]
        m["sidx"] = pre["sidx"][c]
        in_maps.append(m)
    return in_maps


def kernel(x, edge_index, edge_weight, batch, W1, b1, W2, b2, **_unused):
    x = np.asarray(x, dtype=np.float32)
    edge_index = np.asarray(edge_index)
    ew = np.asarray(edge_weight, dtype=np.float64)
    N = x.shape[0]
    row = np.asarray(edge_index[0], dtype=np.int64)
    col = np.asarray(edge_index[1], dtype=np.int64)

    pre = _preprocess(row, col, ew, N)
    nc = _get_program(pre)
    in_maps = _make_inputs(x, W1, b1, W2, b2, pre)
    res = bass_utils.run_bass_kernel_spmd(nc, in_maps,
                                          core_ids=list(range(NCORES)))
    npc, ntile = pre["npc"], pre["ntile"]
    out = np.concatenate([res.results[c]["out"][:npc] for c in range(NCORES)],
                         axis=0)
    return out.astype(np.float32)


# revision 10
# speedup vs baseline: 44.3710x; 3.8926x over previous
"""GCN block (2-layer GCNConv + ReLU) on 8 Trainium2 NeuronCores.

Instruction-count-minimal design (this runtime costs ~40us per
instruction, so the per-edge matmul formulation at ~6700 instructions
is dispatch-bound; this program uses ~180).

Per core (owns 6250 target nodes), per layer:
  - Targets are degree-sorted into 49 blocks of 128 (permutation sigma);
    each target's edges (incl. its self-loop) form a vertical run in a
    [128 channel x chunk] grid: channel = rank%128 within its block,
    chunks = consecutive rows of the block's region. Regions have
    uniform height and a single source-half.
  - Sources are split into lo (table row < 32768) / hi halves so the
    int16 dma_gather indices reach all 50176 padded table rows.
  - Per region (<= 8192 slots): one streaming dma_gather (node-major
    rows, 256B each), one broadcast multiply by the per-edge GCN norm,
    and one strided tensor_reduce over the height axis that lands
    per-target sums directly into a staging tile (fp32).
  - lo + hi stagings are added; layer 1 then applies W1 (+b1, relu) and
    W2 feature-major after a single DMA-transpose read, and writes the
    result node-major via one DVE stream-transpose + 3 block-permuted
    DMA writes.  An AllGather shares the (sigma-permuted) t2 table.
  - Layer 2 needs no weights (W2 folded in); its staged result gets
    +b2, relu, and ONE duplicate-free dma_scatter_add that undoes
    sigma into true node order.
"""

import os
import sys

for _p in ("/opt/trn_rl_repo", "/root/.axon_site/_ro/trn_rl_repo"):
    if os.path.isdir(_p) and _p not in sys.path:
        sys.path.insert(0, _p)

import numpy as np

import concourse.bass as bass
import concourse.bacc as bacc
import concourse.mybir as mybir
import concourse.tile as tile
from concourse import bass_utils

F16 = mybir.dt.float16
F32 = mybir.dt.float32
I16 = mybir.dt.int16

NCORES = 8
D = 96
EW = 128                 # padded row width (fp16 -> 256B)
P = 128
HALF = 32768
REG_CAP = 96             # chunks per region (<= 12288 gather indices)
MMW = 512                # matmul moving width (one PSUM bank)


def _pack_half(dh, rank_order, nblk):
    """Pack one source-half's per-target degrees into regions.

    dh: [ntile] degrees for this half in sigma (rank) order, padded to
    nblk*128.  Returns list of regions (b0, nb, h) and total chunks.
    """
    hb = dh.reshape(nblk, P).max(axis=1)          # block heights
    regions = []
    b = 0
    while b < nblk:
        h = int(hb[b])
        if h == 0:
            b += 1
            continue
        nb = 1
        while b + nb < nblk:
            h2 = max(h, int(hb[b + nb]))
            if hb[b + nb] == 0 or (nb + 1) * h2 > REG_CAP:
                break
            h = h2
            nb += 1
        regions.append((b, nb, h))
        b += nb
    return regions


def _preprocess(row, col, ew, N):
    npc = N // NCORES            # 6250
    nblk = (npc + P - 1) // P    # 49
    ntile = nblk * P             # 6272

    deg = np.bincount(col, weights=ew, minlength=N) + 1.0
    dinv = (1.0 / np.sqrt(deg)).astype(np.float64)
    norm = dinv[row] * ew * dinv[col]
    selfn = dinv * dinv

    core_of = col // npc
    tloc = col - core_of * npc

    # per-core degree (edges + self) of each local target
    d_all = np.zeros((NCORES, ntile), np.int64)
    np.add.at(d_all, (core_of, tloc), 1)
    d_all[:, :npc] += 1          # self-loops
    assert d_all.max() <= REG_CAP, d_all.max()

    # sigma: rank order of targets (degree desc) per core
    rank_order = np.zeros((NCORES, ntile), np.int64)   # slot -> local target
    sigma = np.zeros((NCORES, ntile), np.int64)        # local target -> slot
    for c in range(NCORES):
        o = np.argsort(-d_all[c], kind="stable")
        rank_order[c] = o
        sigma[c, o] = np.arange(ntile)

    # remapped table rows
    r1 = (row // npc) * ntile + (row % npc)            # layer-1 source rows
    r2 = (row // npc) * ntile + sigma[row // npc, row % npc]
    own = np.arange(N)
    r1_self = (own // npc) * ntile + (own % npc)
    r2_self = (own // npc) * ntile + sigma[own // npc, own % npc]

    # per-core, per-layer, per-half edge lists grouped by (slot, rank)
    # layer l edge (src_row_l, slot, norm); half by src_row_l < HALF.
    # Degrees per half (shared structure across layers since src rows of
    # an edge are in the same half?  NOT guaranteed: r1 vs r2 differ ->
    # halves can differ per layer.  Keep per-layer structures.
    pre = {"npc": npc, "nblk": nblk, "ntile": ntile,
           "sigma": sigma, "rank_order": rank_order}

    for L, (rr, rs) in enumerate(((r1, r1_self), (r2, r2_self))):
        # full edge list incl self-loops
        e_src = np.concatenate([rr, rs])
        e_core = np.concatenate([core_of, own // npc])
        e_slot_local = np.concatenate([tloc, own % npc])
        e_slot = sigma[e_core, e_slot_local]
        e_norm = np.concatenate([norm, selfn])
        e_lo = e_src < HALF

        d_h = np.zeros((2, NCORES, ntile), np.int64)
        np.add.at(d_h, ((~e_lo).astype(np.int64), e_core, e_slot), 1)

        # shared region structure: block heights = max over cores
        regions = []
        for h in range(2):
            dh_max = d_h[h].reshape(NCORES, nblk, P).max(axis=(0, 2))
            dh_flat = np.repeat(dh_max, P)
            regions.append(_pack_half(dh_flat, None, nblk))
        tot_chunks = sum(nb * hh for rgs in regions
                         for (_b0, nb, hh) in rgs)

        # chunk base of each block in each half
        chunk_base = np.full((2, nblk), -1, np.int64)
        blk_h = np.zeros((2, nblk), np.int64)
        cb = 0
        reg_list = []   # (half, b0, nb, h, chunk_base)
        for h in range(2):
            for (b0, nb, hh) in regions[h]:
                reg_list.append((h, b0, nb, hh, cb))
                for k in range(nb):
                    chunk_base[h, b0 + k] = cb + k * hh
                    blk_h[h, b0 + k] = hh
                cb += nb * hh
        assert cb == tot_chunks

        # slot position of every edge: rank within (core, half, slot)
        order = np.lexsort((e_norm, e_src, e_slot, e_core, ~e_lo))
        es, ec, eslot, enorm, elo = (e_src[order], e_core[order],
                                     e_slot[order], e_norm[order],
                                     e_lo[order])
        ehalf = (~elo).astype(np.int64)
        key = ((ehalf * NCORES + ec) * ntile + eslot)
        first = np.ones(len(key), bool)
        first[1:] = key[1:] != key[:-1]
        idx_in_run = np.arange(len(key)) - np.maximum.accumulate(
            np.where(first, np.arange(len(key)), 0))
        # chunk of edge = chunk_base[half, block] + rank; channel = slot%128
        eblk = eslot // P
        echunk = chunk_base[ehalf, eblk] + idx_in_run
        assert (idx_in_run < blk_h[ehalf, eblk]).all()
        echan = eslot % P

        # gather idx / norm arrays
        gidx = np.zeros((NCORES, P, tot_chunks * 8), np.int16)
        gnorm = np.zeros((NCORES, P, tot_chunks), np.float16)
        src_rel = np.where(elo, es, es - HALF).astype(np.int64)
        pos = echunk * P + echan
        # wrap: position i -> [16k + i%16, i//16]
        prow = (pos % 16).astype(np.int64)
        pcol = (pos // 16).astype(np.int64)
        for k in range(8):
            gidx[ec, 16 * k + prow, pcol] = src_rel
        gnorm[ec, echan, echunk] = enorm

        pre[f"gidx{L}"] = gidx
        pre[f"gnorm{L}"] = gnorm
        pre[f"regs{L}"] = reg_list
        pre[f"chunks{L}"] = tot_chunks

    # scatter indices (undo sigma; pads -> distinct unused rows)
    sidx = np.zeros((NCORES, P, ntile // 16), np.int16)
    for c in range(NCORES):
        tgt = np.where(rank_order[c] < ntile, rank_order[c], 0)
        # slots holding real targets scatter there; pad slots (target id
        # >= npc would collide) -> unused rows npc..ntile-1, distinct.
        pad_rows = np.arange(npc, ntile)
        is_pad = rank_order[c] >= npc
        tgt = rank_order[c].copy()
        tgt[is_pad] = pad_rows[:is_pad.sum()]
        pos = np.arange(ntile)
        for k in range(8):
            sidx[c, 16 * k + pos % 16, pos // 16] = tgt
    pre["sidx"] = sidx
    return pre


def _build_program(pre, repeat=1, no_coll=False):
    npc, nblk, ntile = pre["npc"], pre["nblk"], pre["ntile"]
    NROWS = NCORES * ntile
    ch0, ch1 = pre["chunks0"], pre["chunks1"]

    nc = bacc.Bacc("TRN2", target_bir_lowering=False, debug=False,
                   enable_asserts=False, num_devices=NCORES,
                   num_swdge_queues=4)

    xpad_d = nc.dram_tensor("xpad", [NROWS, EW], F16, kind="ExternalInput").ap()
    gidx0_d = nc.dram_tensor("gidx0", [P, ch0 * 8], I16, kind="ExternalInput").ap()
    gidx1_d = nc.dram_tensor("gidx1", [P, ch1 * 8], I16, kind="ExternalInput").ap()
    gnorm0_d = nc.dram_tensor("gnorm0", [P, ch0], F16, kind="ExternalInput").ap()
    gnorm1_d = nc.dram_tensor("gnorm1", [P, ch1], F16, kind="ExternalInput").ap()
    sidx_d = nc.dram_tensor("sidx", [P, ntile // 16], I16, kind="ExternalInput").ap()
    w1_d = nc.dram_tensor("w1", [D, D], F16, kind="ExternalInput").ap()
    w2_d = nc.dram_tensor("w2", [D, D], F16, kind="ExternalInput").ap()
    b1_d = nc.dram_tensor("b1", [D, 1], F32, kind="ExternalInput").ap()
    b2rep_d = nc.dram_tensor("b2rep", [P, EW], F32, kind="ExternalInput").ap()
    out_d = nc.dram_tensor("out", [ntile, D], F32, kind="ExternalOutput").ap()

    NSL = 13  # ceil(6272/512) matmul slices

    with tile.TileContext(nc) as tc:
        with (
            tc.tile_pool(name="const", bufs=1) as cpool,
            tc.tile_pool(name="gath", bufs=2) as gpool,
            tc.tile_pool(name="stage", bufs=1) as spool,
            tc.tile_pool(name="work", bufs=1) as wpool,
            tc.tile_pool(name="psum", bufs=4, space="PSUM") as ppool,
            tc.tile_pool(name="dram", bufs=1, space="DRAM") as dpool,
        ):
            # ---- persistent loads (outside the repeated body)
            chmax = max(ch0, ch1)
            gidx_sb = cpool.tile([P, chmax * 8], I16, tag="gidx")
            gnorm_sb = cpool.tile([P, chmax], F16, tag="gnorm")
            sidx_sb = cpool.tile([P, ntile // 16], I16, tag="sidx")
            w1_sb = cpool.tile([D, D], F16, tag="w1")
            w2_sb = cpool.tile([D, D], F16, tag="w2")
            b1_sb = cpool.tile([D, 1], F32, tag="b1")
            b2rep_sb = cpool.tile([P, EW], F32, tag="b2rep")
            zero_sb = cpool.tile([P, nblk * EW], F16, tag="zero")
            for sb, dr in ((sidx_sb, sidx_d), (w1_sb, w1_d), (w2_sb, w2_d),
                           (b1_sb, b1_d), (b2rep_sb, b2rep_d)):
                nc.sync.dma_start(sb[:], dr[:])
            nc.vector.memset(zero_sb[:], 0.0)

            t2_own = dpool.tile([ntile, EW], F16, tag="t2own")
            # zero pad cols once (perm writes only cols :96)
            nc.sync.dma_start(
                t2_own[:].rearrange("(b p) e -> p b e", p=P),
                zero_sb[:].rearrange("p (b e) -> p b e", e=EW))
            t2_fulls = [
                dpool.tile([NROWS, EW], F16, tag=f"t2full{r}",
                           addr_space="Shared", name=f"t2full{r}")
                for r in range(repeat)
            ]
            agg_perm = dpool.tile([ntile, EW], F16, tag="aggperm")
            outf16 = dpool.tile([ntile, EW], F16, tag="outf16")

            def aggregate(L, table, gidx_d_l, gnorm_d_l, ch_l):
                """gather+norm+reduce all regions; returns fp32 stagings."""
                nc.sync.dma_start(gidx_sb[:, :ch_l * 8], gidx_d_l[:])
                nc.sync.dma_start(gnorm_sb[:, :ch_l], gnorm_d_l[:])
                st = [spool.tile([P, nblk * EW], F32, tag=f"stg{h}",
                                 name=f"stg{h}")
                      for h in range(2)]
                nc.vector.memset(st[0][:], 0.0)
                nc.vector.memset(st[1][:], 0.0)
                for ri, (h, b0, nb, hh, cb) in enumerate(pre[f"regs{L}"]):
                    nch = nb * hh
                    ni = nch * P
                    g = gpool.tile([P, REG_CAP * EW], F16, tag="reg",
                                   name="reg")
                    src = table if h == 0 else table[HALF:, :]
                    nc.gpsimd.dma_gather(
                        out_ap=g[:, :nch * EW].rearrange(
                            "p (g e) -> p g e", e=EW),
                        in_ap=src,
                        idxs_ap=gidx_sb[:, cb * 8:(cb + nch) * 8],
                        num_idxs=ni,
                        num_idxs_reg=ni,
                        elem_size=EW,
                        single_packet=False,
                        queue_num=ri % 4,
                    )
                    gv = g[:, :nch * EW].rearrange("p (g e) -> p g e", e=EW)
                    na = gnorm_sb[:, cb:cb + nch]
                    nb_ap = bass.AP(na.tensor, na.offset,
                                    [list(na.ap[0]), list(na.ap[1]), [0, EW]])
                    nc.vector.tensor_tensor(out=gv, in0=gv, in1=nb_ap,
                                            op=mybir.AluOpType.mult)
                    pa = g[:]
                    rin = bass.AP(pa.tensor, pa.offset,
                                  [list(pa.ap[0]), [hh * EW, nb], [1, EW],
                                   [EW, hh]])
                    sa = st[h][:, b0 * EW:(b0 + nb) * EW]
                    rout = bass.AP(sa.tensor, sa.offset,
                                   [list(sa.ap[0]), [EW, nb], [1, EW]])
                    nc.vector.tensor_reduce(out=rout, in_=rin,
                                            axis=mybir.AxisListType.X,
                                            op=mybir.AluOpType.add)
                return st

            for rep in range(repeat):
                t2_full = t2_fulls[rep]

                # ================= layer 1 =================
                st = aggregate(0, xpad_d, gidx0_d, gnorm0_d, ch0)
                agg_sb = wpool.tile([P, nblk * EW], F16, tag="agg16")
                nc.vector.tensor_tensor(out=agg_sb[:], in0=st[0][:],
                                        in1=st[1][:], op=mybir.AluOpType.add)
                nc.sync.dma_start(
                    agg_perm[:].rearrange("(b p) e -> p b e", p=P),
                    agg_sb[:].rearrange("p (b e) -> p b e", e=EW))
                aggT_sb = wpool.tile([P, ntile], F16, tag="big1")
                nc.sync.dma_start(aggT_sb[:], agg_perm[:], transpose=True)

                h1T_sb = wpool.tile([D, ntile], F16, tag="h1T")
                for s in range(NSL):
                    c0, c1 = s * MMW, min((s + 1) * MMW, ntile)
                    pz = ppool.tile([D, MMW], F32, tag="pz", name="pz")
                    nc.tensor.matmul(out=pz[:, :c1 - c0], lhsT=w1_sb[:],
                                     rhs=aggT_sb[:D, c0:c1],
                                     start=True, stop=True)
                    nc.scalar.activation(
                        h1T_sb[:, c0:c1], pz[:, :c1 - c0],
                        mybir.ActivationFunctionType.Relu,
                        bias=b1_sb[:], scale=1.0)
                t2T_sb = wpool.tile([P, ntile], F16, tag="agg16")
                for s in range(NSL):
                    c0, c1 = s * MMW, min((s + 1) * MMW, ntile)
                    pz = ppool.tile([D, MMW], F32, tag="pz2", name="pz2")
                    nc.tensor.matmul(out=pz[:, :c1 - c0], lhsT=w2_sb[:],
                                     rhs=h1T_sb[:, c0:c1],
                                     start=True, stop=True)
                    nc.scalar.activation(
                        t2T_sb[:D, c0:c1], pz[:, :c1 - c0],
                        mybir.ActivationFunctionType.Copy)

                # node-major write: stream-transpose + 3 perm DMAs
                tt_sb = wpool.tile([P, ntile], F16, tag="big1")
                nc.vector.transpose(tt_sb[:D, :], t2T_sb[:D, :])
                NB = ntile // 32
                for bi in range(3):
                    srcap = tt_sb[32 * bi:32 * (bi + 1), :].rearrange(
                        "a (bj b) -> a bj b", b=32)
                    da = t2_own[:]
                    dst = bass.AP(da.tensor, da.offset + 32 * bi,
                                  [[EW, 32], [32 * EW, NB], [1, 32]])
                    nc.sync.dma_start(dst, srcap)

                if not no_coll:
                    nc.gpsimd.collective_compute(
                        "AllGather",
                        mybir.AluOpType.bypass,
                        replica_groups=[list(range(NCORES))],
                        ins=[t2_own[:]],
                        outs=[t2_full[:]],
                    )

                # ================= layer 2 =================
                st = aggregate(1, t2_full[:], gidx1_d, gnorm1_d, ch1)
                nc.vector.tensor_tensor(out=st[0][:], in0=st[0][:],
                                        in1=st[1][:], op=mybir.AluOpType.add)
                b2b = b2rep_sb[:]
                b2_ap = bass.AP(b2b.tensor, b2b.offset,
                                [list(b2b.ap[0]), [0, nblk], [1, EW]])
                nc.vector.tensor_tensor(
                    out=st[0][:].rearrange("p (b e) -> p b e", e=EW),
                    in0=st[0][:].rearrange("p (b e) -> p b e", e=EW),
                    in1=b2_ap, op=mybir.AluOpType.add)
                stg16 = wpool.tile([P, nblk * EW], F16, tag="agg16")
                nc.scalar.activation(stg16[:], st[0][:],
                                     mybir.ActivationFunctionType.Relu)
                # zero the scatter target, then undo sigma in one scatter
                nc.sync.dma_start(
                    outf16[:].rearrange("(b p) e -> p b e", p=P),
                    zero_sb[:].rearrange("p (b e) -> p b e", e=EW))
                nc.gpsimd.dma_scatter_add(
                    out_ap=outf16[:],
                    in_ap=stg16[:].rearrange("p (g e) -> p g e", e=EW),
                    idxs_ap=sidx_sb[:],
                    num_idxs=ntile,
                    num_idxs_reg=ntile,
                    elem_size=EW,
                    single_packet=False,
                )
                nc.gpsimd.dma_start(
                    out_d[:].rearrange("(b p) e -> p b e", p=P),
                    outf16[:].rearrange("(b p) e -> p b e", p=P)[:, :, :D])

    nc.compile()
    return nc


_CACHE = {}


def _get_program(pre, repeat=1, no_coll=False):
    key = (pre["chunks0"], pre["chunks1"], tuple(pre["regs0"]),
           tuple(pre["regs1"]), repeat, no_coll)
    if key not in _CACHE:
        _CACHE[key] = _build_program(pre, repeat=repeat, no_coll=no_coll)
    return _CACHE[key]


def _make_inputs(x, W1, b1, W2, b2, pre):
    npc, ntile = pre["npc"], pre["ntile"]
    N = NCORES * npc
    xpad = np.zeros((NCORES * ntile, EW), np.float16)
    xv = np.asarray(x, np.float32).astype(np.float16)
    for c in range(NCORES):
        xpad[c * ntile:c * ntile + npc, :D] = xv[c * npc:(c + 1) * npc]
    b2rep = np.zeros((P, EW), np.float32)
    b2rep[:, :D] = np.asarray(b2, np.float32)[None, :]
    common = {
        "xpad": xpad,
        "w1": np.asarray(W1, np.float32).astype(np.float16),
        "w2": np.asarray(W2, np.float32).astype(np.float16),
        "b1": np.asarray(b1, np.float32).reshape(D, 1),
        "b2rep": b2rep,
    }
    in_maps = []
    for c in range(NCORES):
        m = dict(common)
        m["gidx0"] = pre["gidx0"][c]
        m["gidx1"] = pre["gidx1"][c]
        m["gnorm0"] = pre["gnorm0"][c]
        m["gnorm1"] = pre["gnorm1"][c]
        m["sidx"] = pre["sidx"][c]
        in_maps.append(m)
    return in_maps


def kernel(x, edge_index, edge_weight, batch, W1, b1, W2, b2, **_unused):
    x = np.asarray(x, dtype=np.float32)
    edge_index = np.asarray(edge_index)
    ew = np.asarray(edge_weight, dtype=np.float64)
    N = x.shape[0]
    row = np.asarray(edge_index[0], dtype=np.int64)
    col = np.asarray(edge_index[1], dtype=np.int64)

    pre = _preprocess(row, col, ew, N)
    nc = _get_program(pre)
    in_maps = _make_inputs(x, W1, b1, W2, b2, pre)
    res = bass_utils.run_bass_kernel_spmd(nc, in_maps,
                                          core_ids=list(range(NCORES)))
    npc, ntile = pre["npc"], pre["ntile"]
    out = np.concatenate([res.results[c]["out"][:npc] for c in range(NCORES)],
                         axis=0)
    return out.astype(np.float32)


# revision 11
# speedup vs baseline: 52.3456x; 1.1797x over previous
"""GCN block (2-layer GCNConv + ReLU) on 8 Trainium2 NeuronCores.

Instruction-count-minimal design (this runtime costs ~40us per
instruction, so the per-edge matmul formulation at ~6700 instructions
is dispatch-bound; this program uses ~180).

Per core (owns 6250 target nodes), per layer:
  - Targets are degree-sorted into 49 blocks of 128 (permutation sigma);
    each target's edges (incl. its self-loop) form a vertical run in a
    [128 channel x chunk] grid: channel = rank%128 within its block,
    chunks = consecutive rows of the block's region. Regions have
    uniform height and a single source-half.
  - Sources are split into lo (table row < 32768) / hi halves so the
    int16 dma_gather indices reach all 50176 padded table rows.
  - Per region (<= 8192 slots): one streaming dma_gather (node-major
    rows, 256B each), one broadcast multiply by the per-edge GCN norm,
    and one strided tensor_reduce over the height axis that lands
    per-target sums directly into a staging tile (fp32).
  - lo + hi stagings are added; layer 1 then applies W1 (+b1, relu) and
    W2 feature-major after a single DMA-transpose read, and writes the
    result node-major via one DVE stream-transpose + 3 block-permuted
    DMA writes.  An AllGather shares the (sigma-permuted) t2 table.
  - Layer 2 needs no weights (W2 folded in); its staged result gets
    +b2, relu, and ONE duplicate-free dma_scatter_add that undoes
    sigma into true node order.
"""

import os
import sys

for _p in ("/opt/trn_rl_repo", "/root/.axon_site/_ro/trn_rl_repo"):
    if os.path.isdir(_p) and _p not in sys.path:
        sys.path.insert(0, _p)

import numpy as np

import concourse.bass as bass
import concourse.bacc as bacc
import concourse.mybir as mybir
import concourse.tile as tile
from concourse import bass_utils

F16 = mybir.dt.float16
F32 = mybir.dt.float32
I16 = mybir.dt.int16

NCORES = 8
D = 96
EW = 128                 # padded row width (fp16 -> 256B)
P = 128
HALF = 32768
REG_CAP = 96             # chunks per region (<= 12288 gather indices)
MMW = 512                # matmul moving width (one PSUM bank)


def _pack_half(dh, rank_order, nblk):
    """Pack one source-half's per-target degrees into regions.

    dh: [ntile] degrees for this half in sigma (rank) order, padded to
    nblk*128.  Returns list of regions (b0, nb, h) and total chunks.
    """
    hb = dh.reshape(nblk, P).max(axis=1)          # block heights
    regions = []
    b = 0
    while b < nblk:
        h = int(hb[b])
        if h == 0:
            b += 1
            continue
        nb = 1
        while b + nb < nblk:
            h2 = max(h, int(hb[b + nb]))
            if hb[b + nb] == 0 or (nb + 1) * h2 > REG_CAP:
                break
            h = h2
            nb += 1
        regions.append((b, nb, h))
        b += nb
    return regions


def _preprocess(row, col, ew, N):
    npc = N // NCORES            # 6250
    nblk = (npc + P - 1) // P    # 49
    ntile = nblk * P             # 6272

    deg = np.bincount(col, weights=ew, minlength=N) + 1.0
    dinv = (1.0 / np.sqrt(deg)).astype(np.float64)
    norm = dinv[row] * ew * dinv[col]
    selfn = dinv * dinv

    core_of = col // npc
    tloc = col - core_of * npc

    # per-core degree (edges + self) of each local target
    d_all = np.zeros((NCORES, ntile), np.int64)
    np.add.at(d_all, (core_of, tloc), 1)
    d_all[:, :npc] += 1          # self-loops
    assert d_all.max() <= REG_CAP, d_all.max()

    # sigma: rank order of targets (degree desc) per core
    rank_order = np.zeros((NCORES, ntile), np.int64)   # slot -> local target
    sigma = np.zeros((NCORES, ntile), np.int64)        # local target -> slot
    for c in range(NCORES):
        o = np.argsort(-d_all[c], kind="stable")
        rank_order[c] = o
        sigma[c, o] = np.arange(ntile)

    # remapped table rows
    r1 = (row // npc) * ntile + (row % npc)            # layer-1 source rows
    r2 = (row // npc) * ntile + sigma[row // npc, row % npc]
    own = np.arange(N)
    r1_self = (own // npc) * ntile + (own % npc)
    r2_self = (own // npc) * ntile + sigma[own // npc, own % npc]

    # per-core, per-layer, per-half edge lists grouped by (slot, rank)
    # layer l edge (src_row_l, slot, norm); half by src_row_l < HALF.
    # Degrees per half (shared structure across layers since src rows of
    # an edge are in the same half?  NOT guaranteed: r1 vs r2 differ ->
    # halves can differ per layer.  Keep per-layer structures.
    pre = {"npc": npc, "nblk": nblk, "ntile": ntile,
           "sigma": sigma, "rank_order": rank_order}

    for L, (rr, rs) in enumerate(((r1, r1_self), (r2, r2_self))):
        # full edge list incl self-loops
        e_src = np.concatenate([rr, rs])
        e_core = np.concatenate([core_of, own // npc])
        e_slot_local = np.concatenate([tloc, own % npc])
        e_slot = sigma[e_core, e_slot_local]
        e_norm = np.concatenate([norm, selfn])
        e_lo = e_src < HALF

        d_h = np.zeros((2, NCORES, ntile), np.int64)
        np.add.at(d_h, ((~e_lo).astype(np.int64), e_core, e_slot), 1)

        # shared region structure: block heights = max over cores
        regions = []
        for h in range(2):
            dh_max = d_h[h].reshape(NCORES, nblk, P).max(axis=(0, 2))
            dh_flat = np.repeat(dh_max, P)
            regions.append(_pack_half(dh_flat, None, nblk))
        tot_chunks = sum(nb * hh for rgs in regions
                         for (_b0, nb, hh) in rgs)

        # chunk base of each block in each half
        chunk_base = np.full((2, nblk), -1, np.int64)
        blk_h = np.zeros((2, nblk), np.int64)
        cb = 0
        reg_list = []   # (half, b0, nb, h, chunk_base)
        for h in range(2):
            for (b0, nb, hh) in regions[h]:
                reg_list.append((h, b0, nb, hh, cb))
                for k in range(nb):
                    chunk_base[h, b0 + k] = cb + k * hh
                    blk_h[h, b0 + k] = hh
                cb += nb * hh
        assert cb == tot_chunks

        # slot position of every edge: rank within (core, half, slot)
        order = np.lexsort((e_norm, e_src, e_slot, e_core, ~e_lo))
        es, ec, eslot, enorm, elo = (e_src[order], e_core[order],
                                     e_slot[order], e_norm[order],
                                     e_lo[order])
        ehalf = (~elo).astype(np.int64)
        key = ((ehalf * NCORES + ec) * ntile + eslot)
        first = np.ones(len(key), bool)
        first[1:] = key[1:] != key[:-1]
        idx_in_run = np.arange(len(key)) - np.maximum.accumulate(
            np.where(first, np.arange(len(key)), 0))
        # chunk of edge = chunk_base[half, block] + rank; channel = slot%128
        eblk = eslot // P
        echunk = chunk_base[ehalf, eblk] + idx_in_run
        assert (idx_in_run < blk_h[ehalf, eblk]).all()
        echan = eslot % P

        # gather idx / norm arrays
        gidx = np.zeros((NCORES, P, tot_chunks * 8), np.int16)
        gnorm = np.zeros((NCORES, P, tot_chunks), np.float16)
        src_rel = np.where(elo, es, es - HALF).astype(np.int64)
        pos = echunk * P + echan
        # wrap: position i -> [16k + i%16, i//16]
        prow = (pos % 16).astype(np.int64)
        pcol = (pos // 16).astype(np.int64)
        for k in range(8):
            gidx[ec, 16 * k + prow, pcol] = src_rel
        gnorm[ec, echan, echunk] = enorm

        pre[f"gidx{L}"] = gidx
        pre[f"gnorm{L}"] = gnorm
        pre[f"regs{L}"] = reg_list
        pre[f"chunks{L}"] = tot_chunks

    # scatter indices (undo sigma; pads -> distinct unused rows)
    sidx = np.zeros((NCORES, P, ntile // 16), np.int16)
    for c in range(NCORES):
        tgt = np.where(rank_order[c] < ntile, rank_order[c], 0)
        # slots holding real targets scatter there; pad slots (target id
        # >= npc would collide) -> unused rows npc..ntile-1, distinct.
        pad_rows = np.arange(npc, ntile)
        is_pad = rank_order[c] >= npc
        tgt = rank_order[c].copy()
        tgt[is_pad] = pad_rows[:is_pad.sum()]
        pos = np.arange(ntile)
        for k in range(8):
            sidx[c, 16 * k + pos % 16, pos // 16] = tgt
    pre["sidx"] = sidx
    return pre


def _build_program(pre, repeat=1, no_coll=False):
    npc, nblk, ntile = pre["npc"], pre["nblk"], pre["ntile"]
    NROWS = NCORES * ntile
    ch0, ch1 = pre["chunks0"], pre["chunks1"]

    nc = bacc.Bacc("TRN2", target_bir_lowering=False, debug=False,
                   enable_asserts=False, num_devices=NCORES,
                   num_swdge_queues=4)

    xpad_d = nc.dram_tensor("xpad", [NROWS, EW], F16, kind="ExternalInput").ap()
    gidx0_d = nc.dram_tensor("gidx0", [P, ch0 * 8], I16, kind="ExternalInput").ap()
    gidx1_d = nc.dram_tensor("gidx1", [P, ch1 * 8], I16, kind="ExternalInput").ap()
    gnorm0_d = nc.dram_tensor("gnorm0", [P, ch0], F16, kind="ExternalInput").ap()
    gnorm1_d = nc.dram_tensor("gnorm1", [P, ch1], F16, kind="ExternalInput").ap()
    sidx_d = nc.dram_tensor("sidx", [P, ntile // 16], I16, kind="ExternalInput").ap()
    w1_d = nc.dram_tensor("w1", [D, D], F16, kind="ExternalInput").ap()
    w2_d = nc.dram_tensor("w2", [D, D], F16, kind="ExternalInput").ap()
    b1_d = nc.dram_tensor("b1", [D, 1], F32, kind="ExternalInput").ap()
    b2rep_d = nc.dram_tensor("b2rep", [P, EW], F32, kind="ExternalInput").ap()
    out_d = nc.dram_tensor("out", [ntile, D], F32, kind="ExternalOutput").ap()

    NSL = 13  # ceil(6272/512) matmul slices

    with tile.TileContext(nc) as tc:
        with (
            tc.tile_pool(name="const", bufs=1) as cpool,
            tc.tile_pool(name="gath", bufs=2) as gpool,
            tc.tile_pool(name="stage", bufs=1) as spool,
            tc.tile_pool(name="work", bufs=1) as wpool,
            tc.tile_pool(name="psum", bufs=4, space="PSUM") as ppool,
            tc.tile_pool(name="dram", bufs=1, space="DRAM") as dpool,
        ):
            # ---- persistent loads (outside the repeated body)
            chmax = max(ch0, ch1)
            gidx_sb = cpool.tile([P, chmax * 8], I16, tag="gidx")
            gnorm_sb = cpool.tile([P, chmax], F16, tag="gnorm")
            sidx_sb = cpool.tile([P, ntile // 16], I16, tag="sidx")
            w1_sb = cpool.tile([D, D], F16, tag="w1")
            w2_sb = cpool.tile([D, D], F16, tag="w2")
            b1_sb = cpool.tile([D, 1], F32, tag="b1")
            b2rep_sb = cpool.tile([P, EW], F32, tag="b2rep")
            zero_sb = cpool.tile([P, nblk * EW], F16, tag="zero")
            for sb, dr in ((sidx_sb, sidx_d), (w1_sb, w1_d), (w2_sb, w2_d),
                           (b1_sb, b1_d), (b2rep_sb, b2rep_d)):
                nc.sync.dma_start(sb[:], dr[:])
            nc.vector.memset(zero_sb[:], 0.0)

            t2_owns = []
            t2_fulls = []
            for r in range(repeat):
                t2o = dpool.tile([ntile, EW], F16, tag=f"t2own{r}",
                                 name=f"t2own{r}")
                # zero pad cols once (perm writes only cols :96)
                nc.sync.dma_start(
                    t2o[:].rearrange("(b p) e -> p b e", p=P),
                    zero_sb[:].rearrange("p (b e) -> p b e", e=EW))
                t2_owns.append(t2o)
                t2_fulls.append(
                    dpool.tile([NROWS, EW], F16, tag=f"t2full{r}",
                               addr_space="Shared", name=f"t2full{r}"))
            agg_perm = dpool.tile([ntile, EW], F16, tag="aggperm")
            outf16 = dpool.tile([ntile, EW], F16, tag="outf16")

            def aggregate(L, table, gidx_d_l, gnorm_d_l, ch_l):
                """gather+norm+reduce all regions; returns fp32 stagings."""
                nc.sync.dma_start(gidx_sb[:, :ch_l * 8], gidx_d_l[:])
                nc.sync.dma_start(gnorm_sb[:, :ch_l], gnorm_d_l[:])
                st = [spool.tile([P, nblk * EW], F32, tag=f"stg{h}",
                                 name=f"stg{h}")
                      for h in range(2)]
                nc.vector.memset(st[0][:], 0.0)
                nc.vector.memset(st[1][:], 0.0)
                for ri, (h, b0, nb, hh, cb) in enumerate(pre[f"regs{L}"]):
                    nch = nb * hh
                    ni = nch * P
                    g = gpool.tile([P, REG_CAP * EW], F16, tag="reg",
                                   name="reg")
                    src = table if h == 0 else table[HALF:, :]
                    nc.gpsimd.dma_gather(
                        out_ap=g[:, :nch * EW].rearrange(
                            "p (g e) -> p g e", e=EW),
                        in_ap=src,
                        idxs_ap=gidx_sb[:, cb * 8:(cb + nch) * 8],
                        num_idxs=ni,
                        num_idxs_reg=ni,
                        elem_size=EW,
                        single_packet=False,
                        queue_num=ri % 4,
                    )
                    gv = g[:, :nch * EW].rearrange("p (g e) -> p g e", e=EW)
                    na = gnorm_sb[:, cb:cb + nch]
                    nb_ap = bass.AP(na.tensor, na.offset,
                                    [list(na.ap[0]), list(na.ap[1]), [0, EW]])
                    nc.vector.tensor_tensor(out=gv, in0=gv, in1=nb_ap,
                                            op=mybir.AluOpType.mult)
                    pa = g[:]
                    rin = bass.AP(pa.tensor, pa.offset,
                                  [list(pa.ap[0]), [hh * EW, nb], [1, EW],
                                   [EW, hh]])
                    sa = st[h][:, b0 * EW:(b0 + nb) * EW]
                    rout = bass.AP(sa.tensor, sa.offset,
                                   [list(sa.ap[0]), [EW, nb], [1, EW]])
                    nc.vector.tensor_reduce(out=rout, in_=rin,
                                            axis=mybir.AxisListType.X,
                                            op=mybir.AluOpType.add)
                return st

            for rep in range(repeat):
                t2_full = t2_fulls[rep]
                t2_own = t2_owns[rep]

                # ================= layer 1 =================
                st = aggregate(0, xpad_d, gidx0_d, gnorm0_d, ch0)
                agg_sb = wpool.tile([P, nblk * EW], F16, tag="agg16")
                nc.vector.tensor_tensor(out=agg_sb[:], in0=st[0][:],
                                        in1=st[1][:], op=mybir.AluOpType.add)
                nc.sync.dma_start(
                    agg_perm[:].rearrange("(b p) e -> p b e", p=P),
                    agg_sb[:].rearrange("p (b e) -> p b e", e=EW))
                aggT_sb = wpool.tile([P, ntile], F16, tag="big1")
                nc.sync.dma_start(aggT_sb[:], agg_perm[:], transpose=True)

                h1T_sb = wpool.tile([D, ntile], F16, tag="h1T")
                for s in range(NSL):
                    c0, c1 = s * MMW, min((s + 1) * MMW, ntile)
                    pz = ppool.tile([D, MMW], F32, tag="pz", name="pz")
                    nc.tensor.matmul(out=pz[:, :c1 - c0], lhsT=w1_sb[:],
                                     rhs=aggT_sb[:D, c0:c1],
                                     start=True, stop=True)
                    nc.scalar.activation(
                        h1T_sb[:, c0:c1], pz[:, :c1 - c0],
                        mybir.ActivationFunctionType.Relu,
                        bias=b1_sb[:], scale=1.0)
                t2T_sb = wpool.tile([P, ntile], F16, tag="agg16")
                for s in range(NSL):
                    c0, c1 = s * MMW, min((s + 1) * MMW, ntile)
                    pz = ppool.tile([D, MMW], F32, tag="pz2", name="pz2")
                    nc.tensor.matmul(out=pz[:, :c1 - c0], lhsT=w2_sb[:],
                                     rhs=h1T_sb[:, c0:c1],
                                     start=True, stop=True)
                    nc.scalar.activation(
                        t2T_sb[:D, c0:c1], pz[:, :c1 - c0],
                        mybir.ActivationFunctionType.Copy)

                # node-major write: stream-transpose + 3 perm DMAs
                tt_sb = wpool.tile([P, ntile], F16, tag="big1")
                nc.vector.transpose(tt_sb[:D, :], t2T_sb[:D, :])
                NB = ntile // 32
                for bi in range(3):
                    srcap = tt_sb[32 * bi:32 * (bi + 1), :].rearrange(
                        "a (bj b) -> a bj b", b=32)
                    da = t2_own[:]
                    dst = bass.AP(da.tensor, da.offset + 32 * bi,
                                  [[EW, 32], [32 * EW, NB], [1, 32]])
                    nc.sync.dma_start(dst, srcap)

                if not no_coll:
                    nc.gpsimd.collective_compute(
                        "AllGather",
                        mybir.AluOpType.bypass,
                        replica_groups=[list(range(NCORES))],
                        ins=[t2_own[:]],
                        outs=[t2_full[:]],
                    )

                # ================= layer 2 =================
                st = aggregate(1, t2_full[:], gidx1_d, gnorm1_d, ch1)
                nc.vector.tensor_tensor(out=st[0][:], in0=st[0][:],
                                        in1=st[1][:], op=mybir.AluOpType.add)
                b2b = b2rep_sb[:]
                b2_ap = bass.AP(b2b.tensor, b2b.offset,
                                [list(b2b.ap[0]), [0, nblk], [1, EW]])
                nc.vector.tensor_tensor(
                    out=st[0][:].rearrange("p (b e) -> p b e", e=EW),
                    in0=st[0][:].rearrange("p (b e) -> p b e", e=EW),
                    in1=b2_ap, op=mybir.AluOpType.add)
                stg16 = wpool.tile([P, nblk * EW], F16, tag="agg16")
                nc.scalar.activation(stg16[:], st[0][:],
                                     mybir.ActivationFunctionType.Relu)
                # zero the scatter target, then undo sigma in one scatter
                nc.sync.dma_start(
                    outf16[:].rearrange("(b p) e -> p b e", p=P),
                    zero_sb[:].rearrange("p (b e) -> p b e", e=EW))
                nc.gpsimd.dma_scatter_add(
                    out_ap=outf16[:],
                    in_ap=stg16[:].rearrange("p (g e) -> p g e", e=EW),
                    idxs_ap=sidx_sb[:],
                    num_idxs=ntile,
                    num_idxs_reg=ntile,
                    elem_size=EW,
                    single_packet=False,
                )
                nc.gpsimd.dma_start(
                    out_d[:].rearrange("(b p) e -> p b e", p=P),
                    outf16[:].rearrange("(b p) e -> p b e", p=P)[:, :, :D])

    nc.compile()
    return nc


_CACHE = {}


def _get_program(pre, repeat=1, no_coll=False):
    key = (pre["chunks0"], pre["chunks1"], tuple(pre["regs0"]),
           tuple(pre["regs1"]), repeat, no_coll)
    if key not in _CACHE:
        _CACHE[key] = _build_program(pre, repeat=repeat, no_coll=no_coll)
    return _CACHE[key]


def _make_inputs(x, W1, b1, W2, b2, pre):
    npc, ntile = pre["npc"], pre["ntile"]
    N = NCORES * npc
    xpad = np.zeros((NCORES * ntile, EW), np.float16)
    xv = np.asarray(x, np.float32).astype(np.float16)
    for c in range(NCORES):
        xpad[c * ntile:c * ntile + npc, :D] = xv[c * npc:(c + 1) * npc]
    b2rep = np.zeros((P, EW), np.float32)
    b2rep[:, :D] = np.asarray(b2, np.float32)[None, :]
    common = {
        "xpad": xpad,
        "w1": np.asarray(W1, np.float32).astype(np.float16),
        "w2": np.asarray(W2, np.float32).astype(np.float16),
        "b1": np.asarray(b1, np.float32).reshape(D, 1),
        "b2rep": b2rep,
    }
    in_maps = []
    for c in range(NCORES):
        m = dict(common)
        m["gidx0"] = pre["gidx0"][c]
        m["gidx1"] = pre["gidx1"][c]
        m["gnorm0"] = pre["gnorm0"][c]
        m["gnorm1"] = pre["gnorm1"][c]
        m["sidx"] = pre["sidx"][c]
        in_maps.append(m)
    return in_maps


def kernel(x, edge_index, edge_weight, batch, W1, b1, W2, b2, **_unused):
    x = np.asarray(x, dtype=np.float32)
    edge_index = np.asarray(edge_index)
    ew = np.asarray(edge_weight, dtype=np.float64)
    N = x.shape[0]
    row = np.asarray(edge_index[0], dtype=np.int64)
    col = np.asarray(edge_index[1], dtype=np.int64)

    pre = _preprocess(row, col, ew, N)
    nc = _get_program(pre)
    in_maps = _make_inputs(x, W1, b1, W2, b2, pre)
    res = bass_utils.run_bass_kernel_spmd(nc, in_maps,
                                          core_ids=list(range(NCORES)))
    npc, ntile = pre["npc"], pre["ntile"]
    out = np.concatenate([res.results[c]["out"][:npc] for c in range(NCORES)],
                         axis=0)
    return out.astype(np.float32)
